# revision 50
# baseline (speedup 1.0000x reference)
"""ResNet BasicBlock (conv3x3-BN-ReLU-conv3x3-BN-add-ReLU) on 8 Trainium2 cores.

Data-parallel over batch: 32 samples -> 4 per core. Per core, each 3x3 conv
uses 1D Winograd F(2,3) along W: the input is transformed on the Vector
engine into 4 planes (t0=d0-d2, t1=d1+d2, t2=d2-d1, t3=d1-d3 over
even/odd column pairs), each conv reduces to 4 planes x 3 kernel rows x 2
input-channel chunks = 24 f16 matmuls of 392 columns per (output-half,
14-row chunk) accumulating in PSUM -- 1.5x fewer tensor cycles than the
direct 18-matmul form. The output transform (y_even=m0+m1+m2,
y_odd=m1-m2-m3) runs on the Vector engine fused with the residual add;
BN scale is folded into the conv weights on host, BN bias + ReLU applied
by the Scalar engine. conv1 of sample s+1 is emitted before conv2 of
sample s so the tensor queue never waits on the inter-conv transforms.
"""
import os
import sys

for _p in ("/opt/trn_rl_repo", "/root/.axon_site/_ro/trn_rl_repo"):
    if os.path.isdir(_p) and _p not in sys.path:
        sys.path.append(_p)

import numpy as np

EPS = 1e-5

S = 4            # samples per core
C = 256
H = W = 56
PH = 58          # padded rows: image rows 1..56
PW = 60          # padded cols: image cols 2..57, ring zeros at 1,58
FLAT = PH * PW   # 3480
TJ = 28          # column pair tiles
PLANE = PH * TJ  # 1624 elements per transform plane
CH = 14          # output rows per chunk
NCH = 4          # chunks per sample
NCOL = CH * TJ   # 392 moving columns per matmul
N_CORES = 8

_CACHE = {}
LAST_RESULT = None


def _build():
    from concourse import bacc
    import concourse.mybir as mybir
    import concourse.tile as tile

    F32 = mybir.dt.float32
    F16 = mybir.dt.float16
    Relu = mybir.ActivationFunctionType.Relu

    nc = bacc.Bacc(None, target_bir_lowering=False)

    x_d = nc.dram_tensor("x", [S, C, PH, PW], F16, kind="ExternalInput")
    # residual, parity-major: [s, ci, p, parity, row, colpair]
    xr_d = nc.dram_tensor("xres", [S, 2, 128, 2, H, TJ], F16, kind="ExternalInput")
    w1_d = nc.dram_tensor("w1t", [128, 3, 4, 2, 256], F16, kind="ExternalInput")
    w2_d = nc.dram_tensor("w2t", [128, 3, 4, 2, 256], F16, kind="ExternalInput")
    b1_d = nc.dram_tensor("b1t", [128, 2], F32, kind="ExternalInput")
    b2_d = nc.dram_tensor("b2t", [128, 2], F32, kind="ExternalInput")
    id_d = nc.dram_tensor("idpm", [128, 256], F16, kind="ExternalInput")
    z_d = nc.dram_tensor("zeros", [128, CH * PW], F16, kind="ExternalInput")
    y_d = nc.dram_tensor("y", [S, C, H, W], F32, kind="ExternalOutput")

    with tile.TileContext(nc) as tc:
        with (
            tc.tile_pool(name="wpool", bufs=1) as wpool,
            tc.tile_pool(name="img", bufs=1) as img,
            tc.tile_pool(name="tmpp", bufs=2) as tmpp,
            tc.tile_pool(name="prep", bufs=2) as prep,
            tc.tile_pool(name="outp", bufs=2) as outp,
            tc.tile_pool(name="ps", bufs=8, space="PSUM") as ps,
        ):
            w_sb = {
                1: wpool.tile([128, 3 * 4 * 2 * 256], F16, name="w1"),
                2: wpool.tile([128, 3 * 4 * 2 * 256], F16, name="w2"),
            }
            b_sb = {
                1: wpool.tile([128, 2], F32, name="b1"),
                2: wpool.tile([128, 2], F32, name="b2"),
            }
            id_sb = wpool.tile([128, 256], F16, name="idpm")
            warm = wpool.tile([128, 4], F32, name="warm")
            xpad = {(b, ci): img.tile([128, FLAT], F16, name=f"xpad{b}_{ci}")
                    for b in range(2) for ci in range(2)}
            xres = {(b, ci): img.tile([128, H * W], F16, name=f"xres{b}_{ci}")
                    for b in range(2) for ci in range(2)}
            # 14-row padded strips for conv1 output (rotating, rings stay 0)
            strip = [img.tile([128, CH * PW], F16, name=f"strip{u}")
                     for u in range(4)]
            tx = {(b, ci): img.tile([128, 4 * PLANE], F16, name=f"tx{b}_{ci}")
                  for b in range(2) for ci in range(2)}
            to = {(b, ci): img.tile([128, 4 * PLANE], F16, name=f"to{b}_{ci}")
                  for b in range(2) for ci in range(2)}

            def pairs(t):
                # [p, row, colpair j, k] with col = 2j + k
                return t.rearrange("p (r j k) -> p r j k", r=PH, k=2)

            def planes(t):
                # [p, plane, row, j]
                return t.rearrange("p (q r j) -> p q r j", q=4, r=PH)

            def transform_in(src, dst, eng=None, rows=(0, PH)):
                e = eng or nc.vector
                a, z = rows
                v = pairs(src)[:, a:z]
                q = planes(dst)[:, :, a:z, :]
                e.tensor_sub(q[:, 0], v[:, :, 0:28, 1], v[:, :, 1:29, 1])
                e.tensor_add(q[:, 1], v[:, :, 1:29, 0], v[:, :, 1:29, 1])
                e.tensor_sub(q[:, 2], v[:, :, 1:29, 1], v[:, :, 1:29, 0])
                e.tensor_sub(q[:, 3], v[:, :, 1:29, 0], v[:, :, 2:30, 0])

            def transform_strip(st, dst, r0, eng):
                # strip rows are padded rows 1+r0 .. 14+r0; steady-state on
                # GpSimd so DVE keeps psum evacuation flowing (DVE for the
                # first, cold-pipeline sample)
                v = st.rearrange("p (r j k) -> p r j k", r=CH, k=2)
                q = planes(dst)[:, :, 1 + r0:1 + r0 + CH, :]
                eng.tensor_sub(q[:, 0], v[:, :, 0:28, 1], v[:, :, 1:29, 1])
                eng.tensor_add(q[:, 1], v[:, :, 1:29, 0], v[:, :, 1:29, 1])
                eng.tensor_sub(q[:, 2], v[:, :, 1:29, 1], v[:, :, 1:29, 0])
                eng.tensor_sub(q[:, 3], v[:, :, 1:29, 0], v[:, :, 2:30, 0])

            def load_x(s):
                for ci in range(2):
                    nc.sync.dma_start(
                        xpad[(s % 2, ci)].rearrange("p (h w) -> p h w", h=PH),
                        x_d[s, ci * 128:(ci + 1) * 128, :, :])

            def load_xres(s):
                b = s % 2
                for ci in range(2):
                    nc.sync.dma_start(
                        xres[(b, ci)].rearrange("p (k r j) -> p k r j",
                                                k=2, r=H),
                        xr_d[s, ci, :, :, :, :])

            # one conv unit: (conv, co, chunk) -> 4 psum planes, 24 matmuls.
            # conv2 also accumulates the residual via identity matmuls:
            # plane0 += I @ x_even, plane3 += (-I) @ x_odd, so the output
            # transform (y_even=m0+m1+m2, y_odd=m1-m2-m3) picks them up free.
            def conv_unit(conv, b, co, c, evac):
                r0 = CH * c
                pts = [ps.tile([128, 512], F32, name="pplane") for _ in range(4)]
                src = tx if conv == 1 else to
                for pl in range(4):
                    for kh in range(3):
                        for ci in range(2):
                            wofs = ((kh * 4 + pl) * 2 + ci) * 256 + co * 128
                            mv = planes(src[(b, ci)])[
                                :, pl, r0 + kh:r0 + kh + CH, :]
                            last = kh == 2 and ci == 1
                            nc.tensor.matmul(
                                pts[pl][:, 0:NCOL],
                                w_sb[conv][:, wofs:wofs + 128],
                                mv,
                                start=(kh == 0 and ci == 0),
                                stop=(last and not
                                      (conv == 2 and pl in (0, 3))),
                            )
                if conv == 2:
                    xw = xres[(b, co)].rearrange(
                        "p (k r j) -> p k r j", k=2, r=H)
                    nc.tensor.matmul(
                        pts[0][:, 0:NCOL], id_sb[:, 0:128],
                        xw[:, 0, r0:r0 + CH, :], start=False, stop=True)
                    nc.tensor.matmul(
                        pts[3][:, 0:NCOL], id_sb[:, 128:256],
                        xw[:, 1, r0:r0 + CH, :], start=False, stop=True)
                evac(b, co, c, r0, pts)

            def rj(t):
                return t.rearrange("p (r j) -> p r j", r=CH)

            Copy = mybir.ActivationFunctionType.Copy

            def evac1(b, co, c, r0, pts):
                m = [rj(pts[pl][:, 0:NCOL]) for pl in range(4)]
                pre = prep.tile([128, CH * 56], F16, name="pre")
                pv = pre.rearrange("p (r j k) -> p r j k", r=CH, k=2)
                # DVE reads at most one PSUM operand per op: stage m1 in SBUF
                c1 = rj(tmpp.tile([128, NCOL], F32, name="c1"))
                nc.scalar.activation(c1, m[1], Copy)
                ta = rj(tmpp.tile([128, NCOL], F32, name="ta"))
                nc.vector.tensor_add(ta, c1, m[0])
                nc.vector.tensor_add(pv[:, :, :, 0], ta, m[2])
                tb = rj(tmpp.tile([128, NCOL], F32, name="tb"))
                nc.vector.tensor_sub(tb, c1, m[2])
                nc.vector.tensor_sub(pv[:, :, :, 1], tb, m[3])
                st = strip[(co * NCH + c) % 4]
                sv = st.rearrange("p (r w) -> p r w", r=CH)
                nc.scalar.activation(
                    sv[:, :, 2:58],
                    pre.rearrange("p (r w) -> p r w", r=CH),
                    Relu, bias=b_sb[1][:, co:co + 1])
                # defer the strip->planes transform by one unit so the
                # engine queue never head-of-line blocks on this ACT write
                pending.append((st, to[(b, co)], r0))
                if len(pending) > 1:
                    transform_strip(*pending.pop(0), strip_eng[0])

            pending = []
            strip_eng = [None]

            def flush_strips():
                while pending:
                    transform_strip(*pending.pop(0), strip_eng[0])

            def evac2(s):
                def ev(b, co, c, r0, pts):
                    m = [rj(pts[pl][:, 0:NCOL]) for pl in range(4)]
                    pre = prep.tile([128, CH * 56], F16, name="pre")
                    pv = pre.rearrange("p (r j k) -> p r j k", r=CH, k=2)
                    c1 = rj(tmpp.tile([128, NCOL], F32, name="c1"))
                    nc.scalar.activation(c1, m[1], Copy)
                    ta = rj(tmpp.tile([128, NCOL], F32, name="ta"))
                    nc.vector.tensor_add(ta, c1, m[0])
                    nc.vector.tensor_add(pv[:, :, :, 0], ta, m[2])
                    tb = rj(tmpp.tile([128, NCOL], F32, name="tb"))
                    nc.vector.tensor_sub(tb, c1, m[2])
                    nc.vector.tensor_sub(pv[:, :, :, 1], tb, m[3])
                    ot = outp.tile([128, CH * 56], F32, name="ot")
                    nc.scalar.activation(
                        ot.rearrange("p (r w) -> p r w", r=CH),
                        pre.rearrange("p (r w) -> p r w", r=CH),
                        Relu, bias=b_sb[2][:, co:co + 1])
                    nc.sync.dma_start(
                        y_d[s, co * 128:(co + 1) * 128, r0:r0 + CH, :],
                        ot.rearrange("p (r w) -> p r w", r=CH))
                return ev

            def conv_pass(conv, s, evac):
                b = s % 2
                for co in range(2):
                    for c in range(NCH):
                        conv_unit(conv, b, co, c, evac)

            # --- startup ---
            nc.sync.dma_start(
                w_sb[1][:, :], w1_d.rearrange("p a b c d -> p (a b c d)"))
            nc.sync.dma_start(b_sb[1][:, :], b1_d[:, :])
            # pull the scalar engine's activation-table load off the critical
            # path (first real ACTIVATE otherwise starts ~12us late)
            nc.vector.memset(warm[:, :], 0.0)
            nc.scalar.activation(warm[:, 0:1], warm[:, 1:2], Relu)
            nc.scalar.activation(warm[:, 2:3], warm[:, 3:4],
                                 mybir.ActivationFunctionType.Copy)
            load_x(0)
            load_xres(0)
            nc.sync.dma_start(
                w_sb[2][:, :], w2_d.rearrange("p a b c d -> p (a b c d)"))
            nc.sync.dma_start(b_sb[2][:, :], b2_d[:, :])
            nc.sync.dma_start(id_sb[:, :], id_d[:, :])
            # strips: zero whole tiles once (rings persist; interiors rewritten)
            for st in strip:
                nc.sync.dma_start(st[:, :], z_d[:, :])
            # to planes: rows 0 and 57 (zero-pad rows) are never written later
            for b in range(2):
                for ci in range(2):
                    q = planes(to[(b, ci)])
                    zsrc = z_d[:, 0:4 * TJ].rearrange("p (a c) -> p a c", a=4)
                    nc.sync.dma_start(q[:, :, 0, :], zsrc)
                    nc.sync.dma_start(q[:, :, 57, :], zsrc)
            for ci in range(2):
                transform_in(xpad[(0, ci)], tx[(0, ci)])
            load_x(1)
            load_xres(1)
            for ci in range(2):
                transform_in(xpad[(1, ci)], tx[(1, ci)])

            # --- main pipeline: conv1(s+1) emitted before conv2(s) ---
            strip_eng[0] = nc.gpsimd
            conv_pass(1, 0, evac1)
            flush_strips()
            for s in range(S):
                if s + 1 < S:
                    conv_pass(1, s + 1, evac1)
                    flush_strips()
                    if s + 2 < S:
                        load_x(s + 2)
                        for ci in range(2):
                            transform_in(xpad[((s + 2) % 2, ci)],
                                         tx[((s + 2) % 2, ci)])
                conv_pass(2, s, evac2(s))
                if s + 2 < S:
                    load_xres(s + 2)

    nc.compile()
    return nc


def _get_nc():
    if "nc" not in _CACHE:
        _CACHE["nc"] = _build()
    return _CACHE["nc"]


def kernel(x, w1, g1, b1, m1, v1, w2, g2, b2, m2, v2):
    global LAST_RESULT
    from concourse import bass_utils

    x = np.asarray(x, dtype=np.float32)
    N = x.shape[0]
    xp = np.zeros((N, C, PH, PW), dtype=np.float16)
    xp[:, :, 1:57, 2:58] = x
    # parity-major residual: [n, ci, p, parity, row, colpair]
    xres = np.ascontiguousarray(
        x.astype(np.float16).reshape(N, 2, 128, H, TJ, 2).transpose(
            0, 1, 2, 5, 3, 4))

    G = np.array([[1, 0, 0], [0.5, 0.5, 0.5], [0.5, -0.5, 0.5], [0, 0, 1]],
                 np.float64)

    def fold(w, g, bb, m, v):
        inv = np.asarray(g, np.float64) / np.sqrt(np.asarray(v, np.float64) + EPS)
        wp = np.asarray(w, np.float64) * inv[:, None, None, None]
        bp = np.asarray(bb, np.float64) - np.asarray(m, np.float64) * inv
        # wt[pp, kh, plane, ci, o] = sum_kw G[plane, kw] * wp[o, ci*128+pp, kh, kw]
        wt = np.einsum("pw,oihw->hpio", G, wp)          # [3, 4, 256i, 256o]
        wt = wt.reshape(3, 4, 2, 128, 256).transpose(3, 0, 1, 2, 4)
        bt = np.ascontiguousarray(bp.reshape(2, 128).T)
        return np.ascontiguousarray(wt).astype(np.float16), bt.astype(np.float32)

    w1t, b1t = fold(w1, g1, b1, m1, v1)
    w2t, b2t = fold(w2, g2, b2, m2, v2)

    zeros = np.zeros((128, CH * PW), dtype=np.float16)
    idpm = np.concatenate(
        [np.eye(128, dtype=np.float16), -np.eye(128, dtype=np.float16)], axis=1)

    nc = _get_nc()
    in_maps = []
    for c in range(N_CORES):
        in_maps.append({
            "x": np.ascontiguousarray(xp[c * S:(c + 1) * S]),
            "xres": np.ascontiguousarray(xres[c * S:(c + 1) * S]),
            "w1t": w1t, "w2t": w2t, "b1t": b1t, "b2t": b2t,
            "zeros": zeros, "idpm": idpm,
        })

    trace = bool(int(os.environ.get("BASS_KERNEL_TRACE", "0")))
    res = bass_utils.run_bass_kernel_spmd(
        nc, in_maps, core_ids=list(range(N_CORES)), trace=trace)
    LAST_RESULT = res
    out = np.concatenate([r["y"] for r in res.results], axis=0)
    return out


# revision 51
# speedup vs baseline: 1.0415x; 1.0415x over previous
"""ResNet BasicBlock (conv3x3-BN-ReLU-conv3x3-BN-add-ReLU) on 8 Trainium2 cores.

Data-parallel over batch: 32 samples -> 4 per core. Per core, each 3x3 conv
uses 1D Winograd F(2,3) along W: the input is transformed on the Vector
engine into 4 planes (t0=d0-d2, t1=d1+d2, t2=d2-d1, t3=d1-d3 over
even/odd column pairs), each conv reduces to 4 planes x 3 kernel rows x 2
input-channel chunks = 24 f16 matmuls of 392 columns per (output-half,
14-row chunk) accumulating in PSUM -- 1.5x fewer tensor cycles than the
direct 18-matmul form. The output transform (y_even=m0+m1+m2,
y_odd=m1-m2-m3) runs on the Vector engine fused with the residual add;
BN scale is folded into the conv weights on host, BN bias + ReLU applied
by the Scalar engine. conv1 of sample s+1 is emitted before conv2 of
sample s so the tensor queue never waits on the inter-conv transforms.
"""
import os
import sys

for _p in ("/opt/trn_rl_repo", "/root/.axon_site/_ro/trn_rl_repo"):
    if os.path.isdir(_p) and _p not in sys.path:
        sys.path.append(_p)

import numpy as np

EPS = 1e-5

S = 4            # samples per core
C = 256
H = W = 56
PH = 58          # padded rows: image rows 1..56
PW = 60          # padded cols: image cols 2..57, ring zeros at 1,58
FLAT = PH * PW   # 3480
TJ = 28          # column pair tiles
PLANE = PH * TJ  # 1624 elements per transform plane
CH = 14          # output rows per chunk
NCH = 4          # chunks per sample
NCOL = CH * TJ   # 392 moving columns per matmul
N_CORES = 8

_CACHE = {}
LAST_RESULT = None


def _build():
    from concourse import bacc
    import concourse.mybir as mybir
    import concourse.tile as tile

    F32 = mybir.dt.float32
    F16 = mybir.dt.float16
    Relu = mybir.ActivationFunctionType.Relu

    nc = bacc.Bacc(None, target_bir_lowering=False)

    x_d = nc.dram_tensor("x", [S, C, PH, PW], F16, kind="ExternalInput")
    # residual, parity-major: [s, ci, p, parity, row, colpair]
    xr_d = nc.dram_tensor("xres", [S, 2, 128, 2, H, TJ], F16, kind="ExternalInput")
    w1_d = nc.dram_tensor("w1t", [128, 3, 4, 2, 256], F16, kind="ExternalInput")
    w2_d = nc.dram_tensor("w2t", [128, 3, 4, 2, 256], F16, kind="ExternalInput")
    b1_d = nc.dram_tensor("b1t", [128, 2], F32, kind="ExternalInput")
    b2_d = nc.dram_tensor("b2t", [128, 2], F32, kind="ExternalInput")
    id_d = nc.dram_tensor("idpm", [128, 256], F16, kind="ExternalInput")
    z_d = nc.dram_tensor("zeros", [128, CH * PW], F16, kind="ExternalInput")
    y_d = nc.dram_tensor("y", [S, C, H, W], F32, kind="ExternalOutput")

    with tile.TileContext(nc) as tc:
        with (
            tc.tile_pool(name="wpool", bufs=1) as wpool,
            tc.tile_pool(name="img", bufs=1) as img,
            tc.tile_pool(name="tmpp", bufs=2) as tmpp,
            tc.tile_pool(name="prep", bufs=2) as prep,
            tc.tile_pool(name="outp", bufs=2) as outp,
            tc.tile_pool(name="ps", bufs=8, space="PSUM") as ps,
        ):
            w_sb = {
                1: wpool.tile([128, 3 * 4 * 2 * 256], F16, name="w1"),
                2: wpool.tile([128, 3 * 4 * 2 * 256], F16, name="w2"),
            }
            b_sb = {
                1: wpool.tile([128, 2], F32, name="b1"),
                2: wpool.tile([128, 2], F32, name="b2"),
            }
            id_sb = wpool.tile([128, 256], F16, name="idpm")
            warm = wpool.tile([128, 4], F32, name="warm")
            xpad = {(b, ci): img.tile([128, FLAT], F16, name=f"xpad{b}_{ci}")
                    for b in range(2) for ci in range(2)}
            xres = {(b, ci): img.tile([128, H * W], F16, name=f"xres{b}_{ci}")
                    for b in range(2) for ci in range(2)}
            # 14-row padded strips for conv1 output (rotating, rings stay 0)
            strip = [img.tile([128, CH * PW], F16, name=f"strip{u}")
                     for u in range(4)]
            tx = {(b, ci): img.tile([128, 4 * PLANE], F16, name=f"tx{b}_{ci}")
                  for b in range(2) for ci in range(2)}
            to = {(b, ci): img.tile([128, 4 * PLANE], F16, name=f"to{b}_{ci}")
                  for b in range(2) for ci in range(2)}

            def pairs(t):
                # [p, row, colpair j, k] with col = 2j + k
                return t.rearrange("p (r j k) -> p r j k", r=PH, k=2)

            def planes(t):
                # [p, plane, row, j]
                return t.rearrange("p (q r j) -> p q r j", q=4, r=PH)

            def transform_in(src, dst, eng=None, rows=(0, PH)):
                e = eng or nc.vector
                a, z = rows
                v = pairs(src)[:, a:z]
                q = planes(dst)[:, :, a:z, :]
                e.tensor_sub(q[:, 0], v[:, :, 0:28, 1], v[:, :, 1:29, 1])
                e.tensor_add(q[:, 1], v[:, :, 1:29, 0], v[:, :, 1:29, 1])
                e.tensor_sub(q[:, 2], v[:, :, 1:29, 1], v[:, :, 1:29, 0])
                e.tensor_sub(q[:, 3], v[:, :, 1:29, 0], v[:, :, 2:30, 0])

            def transform_strip(st, dst, r0, eng):
                # strip rows are padded rows 1+r0 .. 14+r0; steady-state on
                # GpSimd so DVE keeps psum evacuation flowing (DVE for the
                # first, cold-pipeline sample)
                v = st.rearrange("p (r j k) -> p r j k", r=CH, k=2)
                q = planes(dst)[:, :, 1 + r0:1 + r0 + CH, :]
                eng.tensor_sub(q[:, 0], v[:, :, 0:28, 1], v[:, :, 1:29, 1])
                eng.tensor_add(q[:, 1], v[:, :, 1:29, 0], v[:, :, 1:29, 1])
                eng.tensor_sub(q[:, 2], v[:, :, 1:29, 1], v[:, :, 1:29, 0])
                eng.tensor_sub(q[:, 3], v[:, :, 1:29, 0], v[:, :, 2:30, 0])

            def load_x(s):
                for ci in range(2):
                    nc.sync.dma_start(
                        xpad[(s % 2, ci)].rearrange("p (h w) -> p h w", h=PH),
                        x_d[s, ci * 128:(ci + 1) * 128, :, :])

            def load_xres(s):
                b = s % 2
                for ci in range(2):
                    nc.sync.dma_start(
                        xres[(b, ci)].rearrange("p (k r j) -> p k r j",
                                                k=2, r=H),
                        xr_d[s, ci, :, :, :, :])

            # one conv unit: (conv, co, chunk) -> 4 psum planes, 24 matmuls.
            # conv2 also accumulates the residual via identity matmuls:
            # plane0 += I @ x_even, plane3 += (-I) @ x_odd, so the output
            # transform (y_even=m0+m1+m2, y_odd=m1-m2-m3) picks them up free.
            def conv_unit(conv, b, co, c, evac):
                r0 = CH * c
                pts = [ps.tile([128, 512], F32, name="pplane") for _ in range(4)]
                src = tx if conv == 1 else to
                for pl in range(4):
                    for kh in range(3):
                        for ci in range(2):
                            wofs = ((kh * 4 + pl) * 2 + ci) * 256 + co * 128
                            mv = planes(src[(b, ci)])[
                                :, pl, r0 + kh:r0 + kh + CH, :]
                            last = kh == 2 and ci == 1
                            nc.tensor.matmul(
                                pts[pl][:, 0:NCOL],
                                w_sb[conv][:, wofs:wofs + 128],
                                mv,
                                start=(kh == 0 and ci == 0),
                                stop=(last and not
                                      (conv == 2 and pl in (0, 3))),
                            )
                if conv == 2:
                    xw = xres[(b, co)].rearrange(
                        "p (k r j) -> p k r j", k=2, r=H)
                    nc.tensor.matmul(
                        pts[0][:, 0:NCOL], id_sb[:, 0:128],
                        xw[:, 0, r0:r0 + CH, :], start=False, stop=True)
                    nc.tensor.matmul(
                        pts[3][:, 0:NCOL], id_sb[:, 128:256],
                        xw[:, 1, r0:r0 + CH, :], start=False, stop=True)
                evac(b, co, c, r0, pts)

            def rj(t):
                return t.rearrange("p (r j) -> p r j", r=CH)

            Copy = mybir.ActivationFunctionType.Copy

            def evac1(b, co, c, r0, pts):
                m = [rj(pts[pl][:, 0:NCOL]) for pl in range(4)]
                pre = prep.tile([128, CH * 56], F16, name="pre")
                pv = pre.rearrange("p (r j k) -> p r j k", r=CH, k=2)
                # DVE reads at most one PSUM operand per op: stage m1 in SBUF
                c1 = rj(tmpp.tile([128, NCOL], F32, name="c1"))
                nc.scalar.activation(c1, m[1], Copy)
                ta = rj(tmpp.tile([128, NCOL], F32, name="ta"))
                nc.vector.tensor_add(ta, c1, m[0])
                nc.vector.tensor_add(pv[:, :, :, 0], ta, m[2])
                tb = rj(tmpp.tile([128, NCOL], F32, name="tb"))
                nc.vector.tensor_sub(tb, c1, m[2])
                nc.vector.tensor_sub(pv[:, :, :, 1], tb, m[3])
                st = strip[(co * NCH + c) % 4]
                sv = st.rearrange("p (r w) -> p r w", r=CH)
                nc.scalar.activation(
                    sv[:, :, 2:58],
                    pre.rearrange("p (r w) -> p r w", r=CH),
                    Relu, bias=b_sb[1][:, co:co + 1])
                # defer the strip->planes transform by one unit so the
                # engine queue never head-of-line blocks on this ACT write
                pending.append((st, to[(b, co)], r0))
                if len(pending) > 1:
                    transform_strip(*pending.pop(0), strip_eng[0])

            pending = []
            strip_eng = [None]

            def flush_strips():
                while pending:
                    transform_strip(*pending.pop(0), strip_eng[0])

            def evac2(s):
                def ev(b, co, c, r0, pts):
                    m = [rj(pts[pl][:, 0:NCOL]) for pl in range(4)]
                    pre = prep.tile([128, CH * 56], F16, name="pre")
                    pv = pre.rearrange("p (r j k) -> p r j k", r=CH, k=2)
                    c1 = rj(tmpp.tile([128, NCOL], F32, name="c1"))
                    nc.scalar.activation(c1, m[1], Copy)
                    ta = rj(tmpp.tile([128, NCOL], F32, name="ta"))
                    nc.vector.tensor_add(ta, c1, m[0])
                    nc.vector.tensor_add(pv[:, :, :, 0], ta, m[2])
                    tb = rj(tmpp.tile([128, NCOL], F32, name="tb"))
                    nc.vector.tensor_sub(tb, c1, m[2])
                    nc.vector.tensor_sub(pv[:, :, :, 1], tb, m[3])
                    ot = outp.tile([128, CH * 56], F32, name="ot")
                    nc.scalar.activation(
                        ot.rearrange("p (r w) -> p r w", r=CH),
                        pre.rearrange("p (r w) -> p r w", r=CH),
                        Relu, bias=b_sb[2][:, co:co + 1])
                    nc.sync.dma_start(
                        y_d[s, co * 128:(co + 1) * 128, r0:r0 + CH, :],
                        ot.rearrange("p (r w) -> p r w", r=CH))
                return ev

            def conv_pass(conv, s, evac):
                b = s % 2
                for co in range(2):
                    for c in range(NCH):
                        conv_unit(conv, b, co, c, evac)

            # --- startup ---
            nc.sync.dma_start(
                w_sb[1][:, :], w1_d.rearrange("p a b c d -> p (a b c d)"))
            nc.sync.dma_start(b_sb[1][:, :], b1_d[:, :])
            # pull the scalar engine's activation-table load off the critical
            # path (first real ACTIVATE otherwise starts ~12us late)
            nc.vector.memset(warm[:, :], 0.0)
            nc.scalar.activation(warm[:, 0:1], warm[:, 1:2], Relu)
            nc.scalar.activation(warm[:, 2:3], warm[:, 3:4],
                                 mybir.ActivationFunctionType.Copy)
            load_x(0)
            load_xres(0)
            nc.sync.dma_start(
                w_sb[2][:, :], w2_d.rearrange("p a b c d -> p (a b c d)"))
            nc.sync.dma_start(b_sb[2][:, :], b2_d[:, :])
            nc.sync.dma_start(id_sb[:, :], id_d[:, :])
            # strips: zero whole tiles once (rings persist; interiors rewritten)
            for st in strip:
                nc.sync.dma_start(st[:, :], z_d[:, :])
            # to planes: rows 0 and 57 (zero-pad rows) are never written later
            for b in range(2):
                for ci in range(2):
                    q = planes(to[(b, ci)])
                    zsrc = z_d[:, 0:4 * TJ].rearrange("p (a c) -> p a c", a=4)
                    nc.sync.dma_start(q[:, :, 0, :], zsrc)
                    nc.sync.dma_start(q[:, :, 57, :], zsrc)
            for ci in range(2):
                transform_in(xpad[(0, ci)], tx[(0, ci)])

            # --- main pipeline: conv1(s+1) emitted before conv2(s) ---
            strip_eng[0] = nc.gpsimd
            conv_pass(1, 0, evac1)
            flush_strips()
            load_x(1)
            load_xres(1)
            for ci in range(2):
                transform_in(xpad[(1, ci)], tx[(1, ci)])
            for s in range(S):
                if s + 1 < S:
                    conv_pass(1, s + 1, evac1)
                    flush_strips()
                    if s + 2 < S:
                        load_x(s + 2)
                        for ci in range(2):
                            transform_in(xpad[((s + 2) % 2, ci)],
                                         tx[((s + 2) % 2, ci)])
                conv_pass(2, s, evac2(s))
                if s + 2 < S:
                    load_xres(s + 2)

    nc.compile()
    return nc


def _get_nc():
    if "nc" not in _CACHE:
        _CACHE["nc"] = _build()
    return _CACHE["nc"]


def kernel(x, w1, g1, b1, m1, v1, w2, g2, b2, m2, v2):
    global LAST_RESULT
    from concourse import bass_utils

    x = np.asarray(x, dtype=np.float32)
    N = x.shape[0]
    xp = np.zeros((N, C, PH, PW), dtype=np.float16)
    xp[:, :, 1:57, 2:58] = x
    # parity-major residual: [n, ci, p, parity, row, colpair]
    xres = np.ascontiguousarray(
        x.astype(np.float16).reshape(N, 2, 128, H, TJ, 2).transpose(
            0, 1, 2, 5, 3, 4))

    G = np.array([[1, 0, 0], [0.5, 0.5, 0.5], [0.5, -0.5, 0.5], [0, 0, 1]],
                 np.float64)

    def fold(w, g, bb, m, v):
        inv = np.asarray(g, np.float64) / np.sqrt(np.asarray(v, np.float64) + EPS)
        wp = np.asarray(w, np.float64) * inv[:, None, None, None]
        bp = np.asarray(bb, np.float64) - np.asarray(m, np.float64) * inv
        # wt[pp, kh, plane, ci, o] = sum_kw G[plane, kw] * wp[o, ci*128+pp, kh, kw]
        wt = np.einsum("pw,oihw->hpio", G, wp)          # [3, 4, 256i, 256o]
        wt = wt.reshape(3, 4, 2, 128, 256).transpose(3, 0, 1, 2, 4)
        bt = np.ascontiguousarray(bp.reshape(2, 128).T)
        return np.ascontiguousarray(wt).astype(np.float16), bt.astype(np.float32)

    w1t, b1t = fold(w1, g1, b1, m1, v1)
    w2t, b2t = fold(w2, g2, b2, m2, v2)

    zeros = np.zeros((128, CH * PW), dtype=np.float16)
    idpm = np.concatenate(
        [np.eye(128, dtype=np.float16), -np.eye(128, dtype=np.float16)], axis=1)

    nc = _get_nc()
    in_maps = []
    for c in range(N_CORES):
        in_maps.append({
            "x": np.ascontiguousarray(xp[c * S:(c + 1) * S]),
            "xres": np.ascontiguousarray(xres[c * S:(c + 1) * S]),
            "w1t": w1t, "w2t": w2t, "b1t": b1t, "b2t": b2t,
            "zeros": zeros, "idpm": idpm,
        })

    trace = bool(int(os.environ.get("BASS_KERNEL_TRACE", "0")))
    res = bass_utils.run_bass_kernel_spmd(
        nc, in_maps, core_ids=list(range(N_CORES)), trace=trace)
    LAST_RESULT = res
    out = np.concatenate([r["y"] for r in res.results], axis=0)
    return out


# revision 55
# speedup vs baseline: 1.0708x; 1.0282x over previous
"""ResNet BasicBlock (conv3x3-BN-ReLU-conv3x3-BN-add-ReLU) on 8 Trainium2 cores.

Data-parallel over batch: 32 samples -> 4 per core. Per core, each 3x3 conv
uses 1D Winograd F(2,3) along W: the input is transformed on the Vector
engine into 4 planes (t0=d0-d2, t1=d1+d2, t2=d2-d1, t3=d1-d3 over
even/odd column pairs), each conv reduces to 4 planes x 3 kernel rows x 2
input-channel chunks = 24 f16 matmuls of 392 columns per (output-half,
14-row chunk) accumulating in PSUM -- 1.5x fewer tensor cycles than the
direct 18-matmul form. The output transform (y_even=m0+m1+m2,
y_odd=m1-m2-m3) runs on the Vector engine fused with the residual add;
BN scale is folded into the conv weights on host, BN bias + ReLU applied
by the Scalar engine. conv1 of sample s+1 is emitted before conv2 of
sample s so the tensor queue never waits on the inter-conv transforms.
"""
import os
import sys

for _p in ("/opt/trn_rl_repo", "/root/.axon_site/_ro/trn_rl_repo"):
    if os.path.isdir(_p) and _p not in sys.path:
        sys.path.append(_p)

import numpy as np

EPS = 1e-5

S = 4            # samples per core
C = 256
H = W = 56
PH = 58          # padded rows: image rows 1..56
PW = 60          # padded cols: image cols 2..57, ring zeros at 1,58
FLAT = PH * PW   # 3480
TJ = 28          # column pair tiles
PLANE = PH * TJ  # 1624 elements per transform plane
CH = 14          # output rows per chunk
NCH = 4          # chunks per sample
NCOL = CH * TJ   # 392 moving columns per matmul
N_CORES = 8

_CACHE = {}
LAST_RESULT = None

# residual add: "idmm" = identity matmuls into PSUM, "dve" = vector adds
RESIDUAL = os.environ.get("BASS_KERNEL_RESIDUAL", "idmm")


def _build():
    from concourse import bacc
    import concourse.mybir as mybir
    import concourse.tile as tile

    F32 = mybir.dt.float32
    F16 = mybir.dt.float16
    Relu = mybir.ActivationFunctionType.Relu

    nc = bacc.Bacc(None, target_bir_lowering=False)

    x_d = nc.dram_tensor("x", [S, C, PH, PW], F16, kind="ExternalInput")
    # residual, parity-major: [s, ci, p, parity, row, colpair]
    xr_d = nc.dram_tensor("xres", [S, 2, 128, 2, H, TJ], F16, kind="ExternalInput")
    w1_d = nc.dram_tensor("w1t", [128, 3, 4, 2, 256], F16, kind="ExternalInput")
    w2_d = nc.dram_tensor("w2t", [128, 3, 4, 2, 256], F16, kind="ExternalInput")
    b1_d = nc.dram_tensor("b1t", [128, 2], F32, kind="ExternalInput")
    b2_d = nc.dram_tensor("b2t", [128, 2], F32, kind="ExternalInput")
    id_d = nc.dram_tensor("idpm", [128, 256], F16, kind="ExternalInput")
    z_d = nc.dram_tensor("zeros", [128, CH * PW], F16, kind="ExternalInput")
    y_d = nc.dram_tensor("y", [S, C, H, W], F32, kind="ExternalOutput")

    with tile.TileContext(nc) as tc:
        with (
            tc.tile_pool(name="wpool", bufs=1) as wpool,
            tc.tile_pool(name="img", bufs=1) as img,
            tc.tile_pool(name="tmpp", bufs=2) as tmpp,
            tc.tile_pool(name="prep", bufs=2) as prep,
            tc.tile_pool(name="outp", bufs=2) as outp,
            tc.tile_pool(name="ps", bufs=8, space="PSUM") as ps,
        ):
            w_sb = {
                1: wpool.tile([128, 3 * 4 * 2 * 256], F16, name="w1"),
                2: wpool.tile([128, 3 * 4 * 2 * 256], F16, name="w2"),
            }
            b_sb = {
                1: wpool.tile([128, 2], F32, name="b1"),
                2: wpool.tile([128, 2], F32, name="b2"),
            }
            id_sb = wpool.tile([128, 256], F16, name="idpm")
            warm = wpool.tile([128, 4], F32, name="warm")
            xpad = {(b, ci): img.tile([128, FLAT], F16, name=f"xpad{b}_{ci}")
                    for b in range(2) for ci in range(2)}
            xres = {(b, ci): img.tile([128, H * W], F16, name=f"xres{b}_{ci}")
                    for b in range(2) for ci in range(2)}
            # 14-row padded strips for conv1 output (rotating, rings stay 0)
            strip = [img.tile([128, CH * PW], F16, name=f"strip{u}")
                     for u in range(4)]
            tx = {(b, ci): img.tile([128, 4 * PLANE], F16, name=f"tx{b}_{ci}")
                  for b in range(2) for ci in range(2)}
            to = {(b, ci): img.tile([128, 4 * PLANE], F16, name=f"to{b}_{ci}")
                  for b in range(2) for ci in range(2)}

            def pairs(t):
                # [p, row, colpair j, k] with col = 2j + k
                return t.rearrange("p (r j k) -> p r j k", r=PH, k=2)

            def planes(t):
                # [p, plane, row, j]
                return t.rearrange("p (q r j) -> p q r j", q=4, r=PH)

            def transform_in(src, dst, eng=None, rows=(0, PH)):
                e = eng or nc.vector
                a, z = rows
                v = pairs(src)[:, a:z]
                q = planes(dst)[:, :, a:z, :]
                e.tensor_sub(q[:, 0], v[:, :, 0:28, 1], v[:, :, 1:29, 1])
                e.tensor_add(q[:, 1], v[:, :, 1:29, 0], v[:, :, 1:29, 1])
                e.tensor_sub(q[:, 2], v[:, :, 1:29, 1], v[:, :, 1:29, 0])
                e.tensor_sub(q[:, 3], v[:, :, 1:29, 0], v[:, :, 2:30, 0])

            def transform_strip(st, dst, r0, eng):
                # strip rows are padded rows 1+r0 .. 14+r0; steady-state on
                # GpSimd so DVE keeps psum evacuation flowing (DVE for the
                # first, cold-pipeline sample)
                v = st.rearrange("p (r j k) -> p r j k", r=CH, k=2)
                q = planes(dst)[:, :, 1 + r0:1 + r0 + CH, :]
                eng.tensor_sub(q[:, 0], v[:, :, 0:28, 1], v[:, :, 1:29, 1])
                eng.tensor_add(q[:, 1], v[:, :, 1:29, 0], v[:, :, 1:29, 1])
                eng.tensor_sub(q[:, 2], v[:, :, 1:29, 1], v[:, :, 1:29, 0])
                eng.tensor_sub(q[:, 3], v[:, :, 1:29, 0], v[:, :, 2:30, 0])

            def load_x(s):
                for ci in range(2):
                    nc.sync.dma_start(
                        xpad[(s % 2, ci)].rearrange("p (h w) -> p h w", h=PH),
                        x_d[s, ci * 128:(ci + 1) * 128, :, :])

            def load_xres(s):
                b = s % 2
                for ci in range(2):
                    nc.sync.dma_start(
                        xres[(b, ci)].rearrange("p (k r j) -> p k r j",
                                                k=2, r=H),
                        xr_d[s, ci, :, :, :, :])

            # one conv unit: (conv, co, chunk) -> 4 psum planes, 24 matmuls.
            # conv2 also accumulates the residual via identity matmuls:
            # plane0 += I @ x_even, plane3 += (-I) @ x_odd, so the output
            # transform (y_even=m0+m1+m2, y_odd=m1-m2-m3) picks them up free.
            def conv_unit(conv, b, co, c, evac):
                r0 = CH * c
                pts = [ps.tile([128, 512], F32, name="pplane") for _ in range(4)]
                src = tx if conv == 1 else to
                for pl in range(4):
                    for kh in range(3):
                        for ci in range(2):
                            wofs = ((kh * 4 + pl) * 2 + ci) * 256 + co * 128
                            mv = planes(src[(b, ci)])[
                                :, pl, r0 + kh:r0 + kh + CH, :]
                            last = kh == 2 and ci == 1
                            nc.tensor.matmul(
                                pts[pl][:, 0:NCOL],
                                w_sb[conv][:, wofs:wofs + 128],
                                mv,
                                start=(kh == 0 and ci == 0),
                                stop=(last and not
                                      (conv == 2 and RESIDUAL == "idmm"
                                       and pl in (0, 3))),
                            )
                if conv == 2 and RESIDUAL == "idmm":
                    xw = xres[(b, co)].rearrange(
                        "p (k r j) -> p k r j", k=2, r=H)
                    nc.tensor.matmul(
                        pts[0][:, 0:NCOL], id_sb[:, 0:128],
                        xw[:, 0, r0:r0 + CH, :], start=False, stop=True)
                    nc.tensor.matmul(
                        pts[3][:, 0:NCOL], id_sb[:, 128:256],
                        xw[:, 1, r0:r0 + CH, :], start=False, stop=True)
                evac(b, co, c, r0, pts)

            def rj(t):
                return t.rearrange("p (r j) -> p r j", r=CH)

            Copy = mybir.ActivationFunctionType.Copy

            def evac1(b, co, c, r0, pts):
                m = [rj(pts[pl][:, 0:NCOL]) for pl in range(4)]
                pre = prep.tile([128, CH * 56], F16, name="pre")
                pv = pre.rearrange("p (r j k) -> p r j k", r=CH, k=2)
                # DVE reads at most one PSUM operand per op: stage m1 in SBUF
                c1 = rj(tmpp.tile([128, NCOL], F32, name="c1"))
                nc.scalar.activation(c1, m[1], Copy)
                ta = rj(tmpp.tile([128, NCOL], F32, name="ta"))
                nc.vector.tensor_add(ta, c1, m[0])
                nc.vector.tensor_add(pv[:, :, :, 0], ta, m[2])
                tb = rj(tmpp.tile([128, NCOL], F32, name="tb"))
                nc.vector.tensor_sub(tb, c1, m[2])
                nc.vector.tensor_sub(pv[:, :, :, 1], tb, m[3])
                st = strip[(co * NCH + c) % 4]
                sv = st.rearrange("p (r w) -> p r w", r=CH)
                nc.scalar.activation(
                    sv[:, :, 2:58],
                    pre.rearrange("p (r w) -> p r w", r=CH),
                    Relu, bias=b_sb[1][:, co:co + 1])
                # defer the strip->planes transform by one unit so the
                # engine queue never head-of-line blocks on this ACT write
                pending.append((st, to[(b, co)], r0))
                if len(pending) > 1:
                    transform_strip(*pending.pop(0), strip_eng[0])

            pending = []
            strip_eng = [None]

            def flush_strips():
                while pending:
                    transform_strip(*pending.pop(0), strip_eng[0])

            def evac2(s):
                def ev(b, co, c, r0, pts):
                    m = [rj(pts[pl][:, 0:NCOL]) for pl in range(4)]
                    pre = prep.tile([128, CH * 56], F16, name="pre")
                    pv = pre.rearrange("p (r j k) -> p r j k", r=CH, k=2)
                    c1 = rj(tmpp.tile([128, NCOL], F32, name="c1"))
                    nc.scalar.activation(c1, m[1], Copy)
                    ta = rj(tmpp.tile([128, NCOL], F32, name="ta"))
                    nc.vector.tensor_add(ta, c1, m[0])
                    tb = rj(tmpp.tile([128, NCOL], F32, name="tb"))
                    if RESIDUAL == "idmm":
                        nc.vector.tensor_add(pv[:, :, :, 0], ta, m[2])
                        nc.vector.tensor_sub(tb, c1, m[2])
                        nc.vector.tensor_sub(pv[:, :, :, 1], tb, m[3])
                    else:
                        xw = xres[(b, co)].rearrange(
                            "p (k r j) -> p k r j", k=2, r=H)
                        te_ = rj(tmpp.tile([128, NCOL], F16, name="te"))
                        nc.vector.tensor_add(te_, xw[:, 0, r0:r0 + CH, :],
                                             m[2])
                        nc.vector.tensor_add(pv[:, :, :, 0], ta, te_)
                        nc.vector.tensor_sub(tb, c1, m[2])
                        td = rj(tmpp.tile([128, NCOL], F16, name="td"))
                        nc.vector.tensor_sub(td, xw[:, 1, r0:r0 + CH, :],
                                             m[3])
                        nc.vector.tensor_add(pv[:, :, :, 1], tb, td)
                    ot = outp.tile([128, CH * 56], F32, name="ot")
                    nc.scalar.activation(
                        ot.rearrange("p (r w) -> p r w", r=CH),
                        pre.rearrange("p (r w) -> p r w", r=CH),
                        Relu, bias=b_sb[2][:, co:co + 1])
                    nc.sync.dma_start(
                        y_d[s, co * 128:(co + 1) * 128, r0:r0 + CH, :],
                        ot.rearrange("p (r w) -> p r w", r=CH))
                return ev

            def conv_pass(conv, s, evac):
                b = s % 2
                for co in range(2):
                    for c in range(NCH):
                        conv_unit(conv, b, co, c, evac)

            # --- startup ---
            nc.sync.dma_start(
                w_sb[1][:, :], w1_d.rearrange("p a b c d -> p (a b c d)"))
            nc.sync.dma_start(b_sb[1][:, :], b1_d[:, :])
            # pull the scalar engine's activation-table load off the critical
            # path (first real ACTIVATE otherwise starts ~12us late)
            nc.vector.memset(warm[:, :], 0.0)
            nc.scalar.activation(warm[:, 0:1], warm[:, 1:2], Relu)
            nc.scalar.activation(warm[:, 2:3], warm[:, 3:4],
                                 mybir.ActivationFunctionType.Copy)
            load_x(0)
            load_xres(0)
            nc.sync.dma_start(
                w_sb[2][:, :], w2_d.rearrange("p a b c d -> p (a b c d)"))
            nc.sync.dma_start(b_sb[2][:, :], b2_d[:, :])
            nc.sync.dma_start(id_sb[:, :], id_d[:, :])
            # strips: zero whole tiles once (rings persist; interiors rewritten)
            for st in strip:
                nc.sync.dma_start(st[:, :], z_d[:, :])
            # to planes: rows 0 and 57 (zero-pad rows) are never written later
            for b in range(2):
                for ci in range(2):
                    q = planes(to[(b, ci)])
                    zsrc = z_d[:, 0:4 * TJ].rearrange("p (a c) -> p a c", a=4)
                    nc.sync.dma_start(q[:, :, 0, :], zsrc)
                    nc.sync.dma_start(q[:, :, 57, :], zsrc)
            for ci in range(2):
                transform_in(xpad[(0, ci)], tx[(0, ci)])

            # --- main pipeline: conv1(s+1) emitted before conv2(s) ---
            strip_eng[0] = nc.gpsimd
            conv_pass(1, 0, evac1)
            flush_strips()
            load_x(1)
            load_xres(1)
            for ci in range(2):
                transform_in(xpad[(1, ci)], tx[(1, ci)])
            for s in range(S):
                if s + 1 < S:
                    conv_pass(1, s + 1, evac1)
                    flush_strips()
                    if s + 2 < S:
                        load_x(s + 2)
                        for ci in range(2):
                            transform_in(xpad[((s + 2) % 2, ci)],
                                         tx[((s + 2) % 2, ci)])
                conv_pass(2, s, evac2(s))
                if s + 2 < S:
                    load_xres(s + 2)

    nc.compile()
    return nc


def _get_nc():
    if "nc" not in _CACHE:
        _CACHE["nc"] = _build()
    return _CACHE["nc"]


def kernel(x, w1, g1, b1, m1, v1, w2, g2, b2, m2, v2):
    global LAST_RESULT
    from concourse import bass_utils

    x = np.asarray(x, dtype=np.float32)
    N = x.shape[0]
    xp = np.zeros((N, C, PH, PW), dtype=np.float16)
    xp[:, :, 1:57, 2:58] = x
    # parity-major residual: [n, ci, p, parity, row, colpair]
    xres = np.ascontiguousarray(
        x.astype(np.float16).reshape(N, 2, 128, H, TJ, 2).transpose(
            0, 1, 2, 5, 3, 4))

    G = np.array([[1, 0, 0], [0.5, 0.5, 0.5], [0.5, -0.5, 0.5], [0, 0, 1]],
                 np.float64)

    def fold(w, g, bb, m, v):
        inv = np.asarray(g, np.float64) / np.sqrt(np.asarray(v, np.float64) + EPS)
        wp = np.asarray(w, np.float64) * inv[:, None, None, None]
        bp = np.asarray(bb, np.float64) - np.asarray(m, np.float64) * inv
        # wt[pp, kh, plane, ci, o] = sum_kw G[plane, kw] * wp[o, ci*128+pp, kh, kw]
        wt = np.einsum("pw,oihw->hpio", G, wp)          # [3, 4, 256i, 256o]
        wt = wt.reshape(3, 4, 2, 128, 256).transpose(3, 0, 1, 2, 4)
        bt = np.ascontiguousarray(bp.reshape(2, 128).T)
        return np.ascontiguousarray(wt).astype(np.float16), bt.astype(np.float32)

    w1t, b1t = fold(w1, g1, b1, m1, v1)
    w2t, b2t = fold(w2, g2, b2, m2, v2)

    zeros = np.zeros((128, CH * PW), dtype=np.float16)
    idpm = np.concatenate(
        [np.eye(128, dtype=np.float16), -np.eye(128, dtype=np.float16)], axis=1)

    nc = _get_nc()
    in_maps = []
    for c in range(N_CORES):
        in_maps.append({
            "x": np.ascontiguousarray(xp[c * S:(c + 1) * S]),
            "xres": np.ascontiguousarray(xres[c * S:(c + 1) * S]),
            "w1t": w1t, "w2t": w2t, "b1t": b1t, "b2t": b2t,
            "zeros": zeros, "idpm": idpm,
        })

    trace = bool(int(os.environ.get("BASS_KERNEL_TRACE", "0")))
    res = bass_utils.run_bass_kernel_spmd(
        nc, in_maps, core_ids=list(range(N_CORES)), trace=trace)
    LAST_RESULT = res
    out = np.concatenate([r["y"] for r in res.results], axis=0)
    return out


# revision 56
# speedup vs baseline: 1.0766x; 1.0053x over previous
"""ResNet BasicBlock (conv3x3-BN-ReLU-conv3x3-BN-add-ReLU) on 8 Trainium2 cores.

Data-parallel over batch: 32 samples -> 4 per core. Per core, each 3x3 conv
uses 1D Winograd F(2,3) along W: the input is transformed on the Vector
engine into 4 planes (t0=d0-d2, t1=d1+d2, t2=d2-d1, t3=d1-d3 over
even/odd column pairs), each conv reduces to 4 planes x 3 kernel rows x 2
input-channel chunks = 24 f16 matmuls of 392 columns per (output-half,
14-row chunk) accumulating in PSUM -- 1.5x fewer tensor cycles than the
direct 18-matmul form. The output transform (y_even=m0+m1+m2,
y_odd=m1-m2-m3) runs on the Vector engine fused with the residual add;
BN scale is folded into the conv weights on host, BN bias + ReLU applied
by the Scalar engine. conv1 of sample s+1 is emitted before conv2 of
sample s so the tensor queue never waits on the inter-conv transforms.
"""
import os
import sys

for _p in ("/opt/trn_rl_repo", "/root/.axon_site/_ro/trn_rl_repo"):
    if os.path.isdir(_p) and _p not in sys.path:
        sys.path.append(_p)

import numpy as np

EPS = 1e-5

S = 4            # samples per core
C = 256
H = W = 56
PH = 58          # padded rows: image rows 1..56
PW = 60          # padded cols: image cols 2..57, ring zeros at 1,58
FLAT = PH * PW   # 3480
TJ = 28          # column pair tiles
PLANE = PH * TJ  # 1624 elements per transform plane
CH = 14          # output rows per chunk
NCH = 4          # chunks per sample
NCOL = CH * TJ   # 392 moving columns per matmul
N_CORES = 8

_CACHE = {}
LAST_RESULT = None

# residual add: "idmm" = identity matmuls into PSUM, "dve" = vector adds
RESIDUAL = os.environ.get("BASS_KERNEL_RESIDUAL", "dve")


def _build():
    from concourse import bacc
    import concourse.mybir as mybir
    import concourse.tile as tile

    F32 = mybir.dt.float32
    F16 = mybir.dt.float16
    Relu = mybir.ActivationFunctionType.Relu

    nc = bacc.Bacc(None, target_bir_lowering=False)

    x_d = nc.dram_tensor("x", [S, C, PH, PW], F16, kind="ExternalInput")
    # residual, parity-major: [s, ci, p, parity, row, colpair]
    xr_d = nc.dram_tensor("xres", [S, 2, 128, 2, H, TJ], F16, kind="ExternalInput")
    w1_d = nc.dram_tensor("w1t", [128, 3, 4, 2, 256], F16, kind="ExternalInput")
    w2_d = nc.dram_tensor("w2t", [128, 3, 4, 2, 256], F16, kind="ExternalInput")
    b1_d = nc.dram_tensor("b1t", [128, 2], F32, kind="ExternalInput")
    b2_d = nc.dram_tensor("b2t", [128, 2], F32, kind="ExternalInput")
    id_d = nc.dram_tensor("idpm", [128, 256], F16, kind="ExternalInput")
    z_d = nc.dram_tensor("zeros", [128, CH * PW], F16, kind="ExternalInput")
    y_d = nc.dram_tensor("y", [S, C, H, W], F32, kind="ExternalOutput")

    with tile.TileContext(nc) as tc:
        with (
            tc.tile_pool(name="wpool", bufs=1) as wpool,
            tc.tile_pool(name="img", bufs=1) as img,
            tc.tile_pool(name="tmpp", bufs=2) as tmpp,
            tc.tile_pool(name="prep", bufs=2) as prep,
            tc.tile_pool(name="outp", bufs=2) as outp,
            tc.tile_pool(name="ps", bufs=8, space="PSUM") as ps,
        ):
            w_sb = {
                1: wpool.tile([128, 3 * 4 * 2 * 256], F16, name="w1"),
                2: wpool.tile([128, 3 * 4 * 2 * 256], F16, name="w2"),
            }
            b_sb = {
                1: wpool.tile([128, 2], F32, name="b1"),
                2: wpool.tile([128, 2], F32, name="b2"),
            }
            id_sb = wpool.tile([128, 256], F16, name="idpm")
            warm = wpool.tile([128, 4], F32, name="warm")
            xpad = {(b, ci): img.tile([128, FLAT], F16, name=f"xpad{b}_{ci}")
                    for b in range(2) for ci in range(2)}
            xres = {(b, ci): img.tile([128, H * W], F16, name=f"xres{b}_{ci}")
                    for b in range(2) for ci in range(2)}
            # 14-row padded strips for conv1 output (rotating, rings stay 0)
            strip = [img.tile([128, CH * PW], F16, name=f"strip{u}")
                     for u in range(4)]
            tx = {(b, ci): img.tile([128, 4 * PLANE], F16, name=f"tx{b}_{ci}")
                  for b in range(2) for ci in range(2)}
            to = {(b, ci): img.tile([128, 4 * PLANE], F16, name=f"to{b}_{ci}")
                  for b in range(2) for ci in range(2)}

            def pairs(t):
                # [p, row, colpair j, k] with col = 2j + k
                return t.rearrange("p (r j k) -> p r j k", r=PH, k=2)

            def planes(t):
                # [p, plane, row, j]
                return t.rearrange("p (q r j) -> p q r j", q=4, r=PH)

            def transform_in(src, dst, eng=None, rows=(0, PH)):
                e = eng or nc.vector
                a, z = rows
                v = pairs(src)[:, a:z]
                q = planes(dst)[:, :, a:z, :]
                e.tensor_sub(q[:, 0], v[:, :, 0:28, 1], v[:, :, 1:29, 1])
                e.tensor_add(q[:, 1], v[:, :, 1:29, 0], v[:, :, 1:29, 1])
                e.tensor_sub(q[:, 2], v[:, :, 1:29, 1], v[:, :, 1:29, 0])
                e.tensor_sub(q[:, 3], v[:, :, 1:29, 0], v[:, :, 2:30, 0])

            def transform_strip(st, dst, r0, eng):
                # strip rows are padded rows 1+r0 .. 14+r0; steady-state on
                # GpSimd so DVE keeps psum evacuation flowing (DVE for the
                # first, cold-pipeline sample)
                v = st.rearrange("p (r j k) -> p r j k", r=CH, k=2)
                q = planes(dst)[:, :, 1 + r0:1 + r0 + CH, :]
                eng.tensor_sub(q[:, 0], v[:, :, 0:28, 1], v[:, :, 1:29, 1])
                eng.tensor_add(q[:, 1], v[:, :, 1:29, 0], v[:, :, 1:29, 1])
                eng.tensor_sub(q[:, 2], v[:, :, 1:29, 1], v[:, :, 1:29, 0])
                eng.tensor_sub(q[:, 3], v[:, :, 1:29, 0], v[:, :, 2:30, 0])

            def load_x(s):
                for ci in range(2):
                    nc.sync.dma_start(
                        xpad[(s % 2, ci)].rearrange("p (h w) -> p h w", h=PH),
                        x_d[s, ci * 128:(ci + 1) * 128, :, :])

            def load_xres(s):
                b = s % 2
                for ci in range(2):
                    nc.sync.dma_start(
                        xres[(b, ci)].rearrange("p (k r j) -> p k r j",
                                                k=2, r=H),
                        xr_d[s, ci, :, :, :, :])

            # one conv unit: (conv, co, chunk) -> 4 psum planes, 24 matmuls.
            # conv2 also accumulates the residual via identity matmuls:
            # plane0 += I @ x_even, plane3 += (-I) @ x_odd, so the output
            # transform (y_even=m0+m1+m2, y_odd=m1-m2-m3) picks them up free.
            def conv_unit(conv, b, co, c, evac):
                r0 = CH * c
                pts = [ps.tile([128, 512], F32, name="pplane") for _ in range(4)]
                src = tx if conv == 1 else to
                for pl in range(4):
                    for kh in range(3):
                        for ci in range(2):
                            wofs = ((kh * 4 + pl) * 2 + ci) * 256 + co * 128
                            mv = planes(src[(b, ci)])[
                                :, pl, r0 + kh:r0 + kh + CH, :]
                            last = kh == 2 and ci == 1
                            nc.tensor.matmul(
                                pts[pl][:, 0:NCOL],
                                w_sb[conv][:, wofs:wofs + 128],
                                mv,
                                start=(kh == 0 and ci == 0),
                                stop=(last and not
                                      (conv == 2 and RESIDUAL == "idmm"
                                       and pl in (0, 3))),
                            )
                if conv == 2 and RESIDUAL == "idmm":
                    xw = xres[(b, co)].rearrange(
                        "p (k r j) -> p k r j", k=2, r=H)
                    nc.tensor.matmul(
                        pts[0][:, 0:NCOL], id_sb[:, 0:128],
                        xw[:, 0, r0:r0 + CH, :], start=False, stop=True)
                    nc.tensor.matmul(
                        pts[3][:, 0:NCOL], id_sb[:, 128:256],
                        xw[:, 1, r0:r0 + CH, :], start=False, stop=True)
                evac(b, co, c, r0, pts)

            def rj(t):
                return t.rearrange("p (r j) -> p r j", r=CH)

            Copy = mybir.ActivationFunctionType.Copy

            def evac1(b, co, c, r0, pts):
                m = [rj(pts[pl][:, 0:NCOL]) for pl in range(4)]
                pre = prep.tile([128, CH * 56], F16, name="pre")
                pv = pre.rearrange("p (r j k) -> p r j k", r=CH, k=2)
                # DVE reads at most one PSUM operand per op: stage m1 in SBUF
                c1 = rj(tmpp.tile([128, NCOL], F32, name="c1"))
                nc.scalar.activation(c1, m[1], Copy)
                ta = rj(tmpp.tile([128, NCOL], F32, name="ta"))
                nc.vector.tensor_add(ta, c1, m[0])
                nc.vector.tensor_add(pv[:, :, :, 0], ta, m[2])
                tb = rj(tmpp.tile([128, NCOL], F32, name="tb"))
                nc.vector.tensor_sub(tb, c1, m[2])
                nc.vector.tensor_sub(pv[:, :, :, 1], tb, m[3])
                st = strip[(co * NCH + c) % 4]
                sv = st.rearrange("p (r w) -> p r w", r=CH)
                nc.scalar.activation(
                    sv[:, :, 2:58],
                    pre.rearrange("p (r w) -> p r w", r=CH),
                    Relu, bias=b_sb[1][:, co:co + 1])
                # defer the strip->planes transform by one unit so the
                # engine queue never head-of-line blocks on this ACT write
                pending.append((st, to[(b, co)], r0))
                if len(pending) > 1:
                    transform_strip(*pending.pop(0), strip_eng[0])

            pending = []
            strip_eng = [None]

            def flush_strips():
                while pending:
                    transform_strip(*pending.pop(0), strip_eng[0])

            def evac2(s):
                def ev(b, co, c, r0, pts):
                    m = [rj(pts[pl][:, 0:NCOL]) for pl in range(4)]
                    pre = prep.tile([128, CH * 56], F16, name="pre")
                    pv = pre.rearrange("p (r j k) -> p r j k", r=CH, k=2)
                    c1 = rj(tmpp.tile([128, NCOL], F32, name="c1"))
                    nc.scalar.activation(c1, m[1], Copy)
                    ta = rj(tmpp.tile([128, NCOL], F32, name="ta"))
                    nc.vector.tensor_add(ta, c1, m[0])
                    tb = rj(tmpp.tile([128, NCOL], F32, name="tb"))
                    if RESIDUAL == "idmm":
                        nc.vector.tensor_add(pv[:, :, :, 0], ta, m[2])
                        nc.vector.tensor_sub(tb, c1, m[2])
                        nc.vector.tensor_sub(pv[:, :, :, 1], tb, m[3])
                    else:
                        xw = xres[(b, co)].rearrange(
                            "p (k r j) -> p k r j", k=2, r=H)
                        te_ = rj(tmpp.tile([128, NCOL], F16, name="te"))
                        nc.vector.tensor_add(te_, xw[:, 0, r0:r0 + CH, :],
                                             m[2])
                        nc.vector.tensor_add(pv[:, :, :, 0], ta, te_)
                        nc.vector.tensor_sub(tb, c1, m[2])
                        td = rj(tmpp.tile([128, NCOL], F16, name="td"))
                        nc.vector.tensor_sub(td, xw[:, 1, r0:r0 + CH, :],
                                             m[3])
                        nc.vector.tensor_add(pv[:, :, :, 1], tb, td)
                    ot = outp.tile([128, CH * 56], F32, name="ot")
                    nc.scalar.activation(
                        ot.rearrange("p (r w) -> p r w", r=CH),
                        pre.rearrange("p (r w) -> p r w", r=CH),
                        Relu, bias=b_sb[2][:, co:co + 1])
                    nc.sync.dma_start(
                        y_d[s, co * 128:(co + 1) * 128, r0:r0 + CH, :],
                        ot.rearrange("p (r w) -> p r w", r=CH))
                return ev

            def conv_pass(conv, s, evac):
                b = s % 2
                for co in range(2):
                    for c in range(NCH):
                        conv_unit(conv, b, co, c, evac)

            # --- startup ---
            nc.sync.dma_start(
                w_sb[1][:, :], w1_d.rearrange("p a b c d -> p (a b c d)"))
            nc.sync.dma_start(b_sb[1][:, :], b1_d[:, :])
            # pull the scalar engine's activation-table load off the critical
            # path (first real ACTIVATE otherwise starts ~12us late)
            nc.vector.memset(warm[:, :], 0.0)
            nc.scalar.activation(warm[:, 0:1], warm[:, 1:2], Relu)
            nc.scalar.activation(warm[:, 2:3], warm[:, 3:4],
                                 mybir.ActivationFunctionType.Copy)
            load_x(0)
            load_xres(0)
            nc.sync.dma_start(
                w_sb[2][:, :], w2_d.rearrange("p a b c d -> p (a b c d)"))
            nc.sync.dma_start(b_sb[2][:, :], b2_d[:, :])
            nc.sync.dma_start(id_sb[:, :], id_d[:, :])
            # strips: zero whole tiles once (rings persist; interiors rewritten)
            for st in strip:
                nc.sync.dma_start(st[:, :], z_d[:, :])
            # to planes: rows 0 and 57 (zero-pad rows) are never written later
            for b in range(2):
                for ci in range(2):
                    q = planes(to[(b, ci)])
                    zsrc = z_d[:, 0:4 * TJ].rearrange("p (a c) -> p a c", a=4)
                    nc.sync.dma_start(q[:, :, 0, :], zsrc)
                    nc.sync.dma_start(q[:, :, 57, :], zsrc)
            for ci in range(2):
                transform_in(xpad[(0, ci)], tx[(0, ci)])

            # --- main pipeline: conv1(s+1) emitted before conv2(s) ---
            strip_eng[0] = nc.gpsimd
            conv_pass(1, 0, evac1)
            flush_strips()
            load_x(1)
            load_xres(1)
            for ci in range(2):
                transform_in(xpad[(1, ci)], tx[(1, ci)])
            for s in range(S):
                if s + 1 < S:
                    conv_pass(1, s + 1, evac1)
                    flush_strips()
                    if s + 2 < S:
                        load_x(s + 2)
                        for ci in range(2):
                            transform_in(xpad[((s + 2) % 2, ci)],
                                         tx[((s + 2) % 2, ci)])
                conv_pass(2, s, evac2(s))
                if s + 2 < S:
                    load_xres(s + 2)

    nc.compile()
    return nc


def _get_nc():
    if "nc" not in _CACHE:
        _CACHE["nc"] = _build()
    return _CACHE["nc"]


def kernel(x, w1, g1, b1, m1, v1, w2, g2, b2, m2, v2):
    global LAST_RESULT
    from concourse import bass_utils

    x = np.asarray(x, dtype=np.float32)
    N = x.shape[0]
    xp = np.zeros((N, C, PH, PW), dtype=np.float16)
    xp[:, :, 1:57, 2:58] = x
    # parity-major residual: [n, ci, p, parity, row, colpair]
    xres = np.ascontiguousarray(
        x.astype(np.float16).reshape(N, 2, 128, H, TJ, 2).transpose(
            0, 1, 2, 5, 3, 4))

    G = np.array([[1, 0, 0], [0.5, 0.5, 0.5], [0.5, -0.5, 0.5], [0, 0, 1]],
                 np.float64)

    def fold(w, g, bb, m, v):
        inv = np.asarray(g, np.float64) / np.sqrt(np.asarray(v, np.float64) + EPS)
        wp = np.asarray(w, np.float64) * inv[:, None, None, None]
        bp = np.asarray(bb, np.float64) - np.asarray(m, np.float64) * inv
        # wt[pp, kh, plane, ci, o] = sum_kw G[plane, kw] * wp[o, ci*128+pp, kh, kw]
        wt = np.einsum("pw,oihw->hpio", G, wp)          # [3, 4, 256i, 256o]
        wt = wt.reshape(3, 4, 2, 128, 256).transpose(3, 0, 1, 2, 4)
        bt = np.ascontiguousarray(bp.reshape(2, 128).T)
        return np.ascontiguousarray(wt).astype(np.float16), bt.astype(np.float32)

    w1t, b1t = fold(w1, g1, b1, m1, v1)
    w2t, b2t = fold(w2, g2, b2, m2, v2)

    zeros = np.zeros((128, CH * PW), dtype=np.float16)
    idpm = np.concatenate(
        [np.eye(128, dtype=np.float16), -np.eye(128, dtype=np.float16)], axis=1)

    nc = _get_nc()
    in_maps = []
    for c in range(N_CORES):
        in_maps.append({
            "x": np.ascontiguousarray(xp[c * S:(c + 1) * S]),
            "xres": np.ascontiguousarray(xres[c * S:(c + 1) * S]),
            "w1t": w1t, "w2t": w2t, "b1t": b1t, "b2t": b2t,
            "zeros": zeros, "idpm": idpm,
        })

    trace = bool(int(os.environ.get("BASS_KERNEL_TRACE", "0")))
    res = bass_utils.run_bass_kernel_spmd(
        nc, in_maps, core_ids=list(range(N_CORES)), trace=trace)
    LAST_RESULT = res
    out = np.concatenate([r["y"] for r in res.results], axis=0)
    return out


# revision 57
# speedup vs baseline: 1.0851x; 1.0079x over previous
"""ResNet BasicBlock (conv3x3-BN-ReLU-conv3x3-BN-add-ReLU) on 8 Trainium2 cores.

Data-parallel over batch: 32 samples -> 4 per core. Per core, each 3x3 conv
uses 1D Winograd F(2,3) along W: the input is transformed into 4 planes
(t0=d0-d2, t1=d1+d2, t2=d2-d1, t3=d1-d3 over even/odd column pairs), so
each conv is 4 planes x 3 kernel rows x 2 input-channel chunks = 24 f16
matmuls of 392 columns per (output-half, 14-row chunk), one PSUM bank per
plane -- 1.5x fewer tensor cycles than the direct 18-matmul form. The
output transform (y_even=m0+m1+m2, y_odd=m1-m2-m3) runs on the Vector
engine (plane m1 staged to SBUF by a Scalar-engine copy since DVE reads
at most one PSUM operand), fused with the residual add; BN scale is
folded into the conv weights on host, BN bias + ReLU applied by the
Scalar engine. Engine balance: conv1's output strips are re-transformed
for conv2 on GpSimd; input transforms for sample s+2 run on DVE behind
the evacuation ops; conv1 of sample s+1 is emitted before conv2 of
sample s so the tensor queue never waits on the inter-conv transforms.
Measured ~295us vs the 412us direct-conv f16 baseline (tensor engine 96%
occupied within its span; DVE ~252us, GpSimd ~145us, Scalar ~98us).
"""
import os
import sys

for _p in ("/opt/trn_rl_repo", "/root/.axon_site/_ro/trn_rl_repo"):
    if os.path.isdir(_p) and _p not in sys.path:
        sys.path.append(_p)

import numpy as np

EPS = 1e-5

S = 4            # samples per core
C = 256
H = W = 56
PH = 58          # padded rows: image rows 1..56
PW = 60          # padded cols: image cols 2..57, ring zeros at 1,58
FLAT = PH * PW   # 3480
TJ = 28          # column pair tiles
PLANE = PH * TJ  # 1624 elements per transform plane
CH = 14          # output rows per chunk
NCH = 4          # chunks per sample
NCOL = CH * TJ   # 392 moving columns per matmul
N_CORES = 8

_CACHE = {}
LAST_RESULT = None

# residual add: "idmm" = identity matmuls into PSUM, "dve" = vector adds
RESIDUAL = os.environ.get("BASS_KERNEL_RESIDUAL", "dve")


def _build():
    from concourse import bacc
    import concourse.mybir as mybir
    import concourse.tile as tile

    F32 = mybir.dt.float32
    F16 = mybir.dt.float16
    Relu = mybir.ActivationFunctionType.Relu

    nc = bacc.Bacc(None, target_bir_lowering=False)

    x_d = nc.dram_tensor("x", [S, C, PH, PW], F16, kind="ExternalInput")
    # residual, parity-major: [s, ci, p, parity, row, colpair]
    xr_d = nc.dram_tensor("xres", [S, 2, 128, 2, H, TJ], F16, kind="ExternalInput")
    w1_d = nc.dram_tensor("w1t", [128, 3, 4, 2, 256], F16, kind="ExternalInput")
    w2_d = nc.dram_tensor("w2t", [128, 3, 4, 2, 256], F16, kind="ExternalInput")
    b1_d = nc.dram_tensor("b1t", [128, 2], F32, kind="ExternalInput")
    b2_d = nc.dram_tensor("b2t", [128, 2], F32, kind="ExternalInput")
    id_d = nc.dram_tensor("idpm", [128, 256], F16, kind="ExternalInput")
    z_d = nc.dram_tensor("zeros", [128, CH * PW], F16, kind="ExternalInput")
    y_d = nc.dram_tensor("y", [S, C, H, W], F32, kind="ExternalOutput")

    with tile.TileContext(nc) as tc:
        with (
            tc.tile_pool(name="wpool", bufs=1) as wpool,
            tc.tile_pool(name="img", bufs=1) as img,
            tc.tile_pool(name="tmpp", bufs=2) as tmpp,
            tc.tile_pool(name="prep", bufs=2) as prep,
            tc.tile_pool(name="outp", bufs=2) as outp,
            tc.tile_pool(name="ps", bufs=8, space="PSUM") as ps,
        ):
            w_sb = {
                1: wpool.tile([128, 3 * 4 * 2 * 256], F16, name="w1"),
                2: wpool.tile([128, 3 * 4 * 2 * 256], F16, name="w2"),
            }
            b_sb = {
                1: wpool.tile([128, 2], F32, name="b1"),
                2: wpool.tile([128, 2], F32, name="b2"),
            }
            id_sb = wpool.tile([128, 256], F16, name="idpm")
            warm = wpool.tile([128, 4], F32, name="warm")
            xpad = {(b, ci): img.tile([128, FLAT], F16, name=f"xpad{b}_{ci}")
                    for b in range(2) for ci in range(2)}
            xres = {(b, ci): img.tile([128, H * W], F16, name=f"xres{b}_{ci}")
                    for b in range(2) for ci in range(2)}
            # 14-row padded strips for conv1 output (rotating, rings stay 0)
            strip = [img.tile([128, CH * PW], F16, name=f"strip{u}")
                     for u in range(4)]
            tx = {(b, ci): img.tile([128, 4 * PLANE], F16, name=f"tx{b}_{ci}")
                  for b in range(2) for ci in range(2)}
            to = {(b, ci): img.tile([128, 4 * PLANE], F16, name=f"to{b}_{ci}")
                  for b in range(2) for ci in range(2)}

            def pairs(t):
                # [p, row, colpair j, k] with col = 2j + k
                return t.rearrange("p (r j k) -> p r j k", r=PH, k=2)

            def planes(t):
                # [p, plane, row, j]
                return t.rearrange("p (q r j) -> p q r j", q=4, r=PH)

            def transform_in(src, dst, eng=None, rows=(0, PH)):
                e = eng or nc.vector
                a, z = rows
                v = pairs(src)[:, a:z]
                q = planes(dst)[:, :, a:z, :]
                e.tensor_sub(q[:, 0], v[:, :, 0:28, 1], v[:, :, 1:29, 1])
                e.tensor_add(q[:, 1], v[:, :, 1:29, 0], v[:, :, 1:29, 1])
                e.tensor_sub(q[:, 2], v[:, :, 1:29, 1], v[:, :, 1:29, 0])
                e.tensor_sub(q[:, 3], v[:, :, 1:29, 0], v[:, :, 2:30, 0])

            def transform_strip(st, dst, r0, eng):
                # strip rows are padded rows 1+r0 .. 14+r0; steady-state on
                # GpSimd so DVE keeps psum evacuation flowing (DVE for the
                # first, cold-pipeline sample)
                v = st.rearrange("p (r j k) -> p r j k", r=CH, k=2)
                q = planes(dst)[:, :, 1 + r0:1 + r0 + CH, :]
                eng.tensor_sub(q[:, 0], v[:, :, 0:28, 1], v[:, :, 1:29, 1])
                eng.tensor_add(q[:, 1], v[:, :, 1:29, 0], v[:, :, 1:29, 1])
                eng.tensor_sub(q[:, 2], v[:, :, 1:29, 1], v[:, :, 1:29, 0])
                eng.tensor_sub(q[:, 3], v[:, :, 1:29, 0], v[:, :, 2:30, 0])

            def load_x(s):
                for ci in range(2):
                    nc.sync.dma_start(
                        xpad[(s % 2, ci)].rearrange("p (h w) -> p h w", h=PH),
                        x_d[s, ci * 128:(ci + 1) * 128, :, :])

            def load_xres(s):
                b = s % 2
                for ci in range(2):
                    nc.sync.dma_start(
                        xres[(b, ci)].rearrange("p (k r j) -> p k r j",
                                                k=2, r=H),
                        xr_d[s, ci, :, :, :, :])

            # one conv unit: (conv, co, chunk) -> 4 psum planes, 24 matmuls.
            # conv2 also accumulates the residual via identity matmuls:
            # plane0 += I @ x_even, plane3 += (-I) @ x_odd, so the output
            # transform (y_even=m0+m1+m2, y_odd=m1-m2-m3) picks them up free.
            def conv_unit(conv, b, co, c, evac):
                r0 = CH * c
                pts = [ps.tile([128, 512], F32, name="pplane") for _ in range(4)]
                src = tx if conv == 1 else to
                for pl in range(4):
                    for kh in range(3):
                        for ci in range(2):
                            wofs = ((kh * 4 + pl) * 2 + ci) * 256 + co * 128
                            mv = planes(src[(b, ci)])[
                                :, pl, r0 + kh:r0 + kh + CH, :]
                            last = kh == 2 and ci == 1
                            nc.tensor.matmul(
                                pts[pl][:, 0:NCOL],
                                w_sb[conv][:, wofs:wofs + 128],
                                mv,
                                start=(kh == 0 and ci == 0),
                                stop=(last and not
                                      (conv == 2 and RESIDUAL == "idmm"
                                       and pl in (0, 3))),
                            )
                if conv == 2 and RESIDUAL == "idmm":
                    xw = xres[(b, co)].rearrange(
                        "p (k r j) -> p k r j", k=2, r=H)
                    nc.tensor.matmul(
                        pts[0][:, 0:NCOL], id_sb[:, 0:128],
                        xw[:, 0, r0:r0 + CH, :], start=False, stop=True)
                    nc.tensor.matmul(
                        pts[3][:, 0:NCOL], id_sb[:, 128:256],
                        xw[:, 1, r0:r0 + CH, :], start=False, stop=True)
                evac(b, co, c, r0, pts)

            def rj(t):
                return t.rearrange("p (r j) -> p r j", r=CH)

            Copy = mybir.ActivationFunctionType.Copy

            def evac1(b, co, c, r0, pts):
                m = [rj(pts[pl][:, 0:NCOL]) for pl in range(4)]
                pre = prep.tile([128, CH * 56], F16, name="pre")
                pv = pre.rearrange("p (r j k) -> p r j k", r=CH, k=2)
                # DVE reads at most one PSUM operand per op: stage m1 in SBUF
                c1 = rj(tmpp.tile([128, NCOL], F32, name="c1"))
                nc.scalar.activation(c1, m[1], Copy)
                ta = rj(tmpp.tile([128, NCOL], F32, name="ta"))
                nc.vector.tensor_add(ta, c1, m[0])
                nc.vector.tensor_add(pv[:, :, :, 0], ta, m[2])
                tb = rj(tmpp.tile([128, NCOL], F32, name="tb"))
                nc.vector.tensor_sub(tb, c1, m[2])
                nc.vector.tensor_sub(pv[:, :, :, 1], tb, m[3])
                st = strip[(co * NCH + c) % 4]
                sv = st.rearrange("p (r w) -> p r w", r=CH)
                nc.scalar.activation(
                    sv[:, :, 2:58],
                    pre.rearrange("p (r w) -> p r w", r=CH),
                    Relu, bias=b_sb[1][:, co:co + 1])
                # defer the strip->planes transform by one unit so the
                # engine queue never head-of-line blocks on this ACT write
                pending.append((st, to[(b, co)], r0))
                if len(pending) > 1:
                    transform_strip(*pending.pop(0), strip_eng[0])

            pending = []
            strip_eng = [None]

            def flush_strips():
                while pending:
                    transform_strip(*pending.pop(0), strip_eng[0])

            def evac2(s):
                def ev(b, co, c, r0, pts):
                    m = [rj(pts[pl][:, 0:NCOL]) for pl in range(4)]
                    pre = prep.tile([128, CH * 56], F16, name="pre")
                    pv = pre.rearrange("p (r j k) -> p r j k", r=CH, k=2)
                    c1 = rj(tmpp.tile([128, NCOL], F32, name="c1"))
                    nc.scalar.activation(c1, m[1], Copy)
                    ta = rj(tmpp.tile([128, NCOL], F32, name="ta"))
                    nc.vector.tensor_add(ta, c1, m[0])
                    tb = rj(tmpp.tile([128, NCOL], F32, name="tb"))
                    if RESIDUAL == "idmm":
                        nc.vector.tensor_add(pv[:, :, :, 0], ta, m[2])
                        nc.vector.tensor_sub(tb, c1, m[2])
                        nc.vector.tensor_sub(pv[:, :, :, 1], tb, m[3])
                    else:
                        xw = xres[(b, co)].rearrange(
                            "p (k r j) -> p k r j", k=2, r=H)
                        te_ = rj(tmpp.tile([128, NCOL], F16, name="te"))
                        nc.vector.tensor_add(te_, xw[:, 0, r0:r0 + CH, :],
                                             m[2])
                        nc.vector.tensor_add(pv[:, :, :, 0], ta, te_)
                        nc.vector.tensor_sub(tb, c1, m[2])
                        td = rj(tmpp.tile([128, NCOL], F16, name="td"))
                        nc.vector.tensor_sub(td, xw[:, 1, r0:r0 + CH, :],
                                             m[3])
                        nc.vector.tensor_add(pv[:, :, :, 1], tb, td)
                    ot = outp.tile([128, CH * 56], F32, name="ot")
                    nc.scalar.activation(
                        ot.rearrange("p (r w) -> p r w", r=CH),
                        pre.rearrange("p (r w) -> p r w", r=CH),
                        Relu, bias=b_sb[2][:, co:co + 1])
                    nc.sync.dma_start(
                        y_d[s, co * 128:(co + 1) * 128, r0:r0 + CH, :],
                        ot.rearrange("p (r w) -> p r w", r=CH))
                return ev

            def conv_pass(conv, s, evac):
                b = s % 2
                for co in range(2):
                    for c in range(NCH):
                        conv_unit(conv, b, co, c, evac)

            # --- startup ---
            nc.sync.dma_start(
                w_sb[1][:, :], w1_d.rearrange("p a b c d -> p (a b c d)"))
            nc.sync.dma_start(b_sb[1][:, :], b1_d[:, :])
            # pull the scalar engine's activation-table load off the critical
            # path (first real ACTIVATE otherwise starts ~12us late)
            nc.vector.memset(warm[:, :], 0.0)
            nc.scalar.activation(warm[:, 0:1], warm[:, 1:2], Relu)
            nc.scalar.activation(warm[:, 2:3], warm[:, 3:4],
                                 mybir.ActivationFunctionType.Copy)
            load_x(0)
            load_xres(0)
            nc.sync.dma_start(
                w_sb[2][:, :], w2_d.rearrange("p a b c d -> p (a b c d)"))
            nc.sync.dma_start(b_sb[2][:, :], b2_d[:, :])
            nc.sync.dma_start(id_sb[:, :], id_d[:, :])
            # strips: zero whole tiles once (rings persist; interiors rewritten)
            for st in strip:
                nc.sync.dma_start(st[:, :], z_d[:, :])
            # to planes: rows 0 and 57 (zero-pad rows) are never written later
            for b in range(2):
                for ci in range(2):
                    q = planes(to[(b, ci)])
                    zsrc = z_d[:, 0:4 * TJ].rearrange("p (a c) -> p a c", a=4)
                    nc.sync.dma_start(q[:, :, 0, :], zsrc)
                    nc.sync.dma_start(q[:, :, 57, :], zsrc)
            for ci in range(2):
                transform_in(xpad[(0, ci)], tx[(0, ci)])

            # --- main pipeline: conv1(s+1) emitted before conv2(s) ---
            strip_eng[0] = nc.gpsimd
            conv_pass(1, 0, evac1)
            flush_strips()
            load_x(1)
            load_xres(1)
            for ci in range(2):
                transform_in(xpad[(1, ci)], tx[(1, ci)])
            for s in range(S):
                if s + 1 < S:
                    conv_pass(1, s + 1, evac1)
                    flush_strips()
                    if s + 2 < S:
                        load_x(s + 2)
                        for ci in range(2):
                            transform_in(xpad[((s + 2) % 2, ci)],
                                         tx[((s + 2) % 2, ci)])
                conv_pass(2, s, evac2(s))
                if s + 2 < S:
                    load_xres(s + 2)

    nc.compile()
    return nc


def _get_nc():
    if "nc" not in _CACHE:
        _CACHE["nc"] = _build()
    return _CACHE["nc"]


def kernel(x, w1, g1, b1, m1, v1, w2, g2, b2, m2, v2):
    global LAST_RESULT
    from concourse import bass_utils

    x = np.asarray(x, dtype=np.float32)
    N = x.shape[0]
    xp = np.zeros((N, C, PH, PW), dtype=np.float16)
    xp[:, :, 1:57, 2:58] = x
    # parity-major residual: [n, ci, p, parity, row, colpair]
    xres = np.ascontiguousarray(
        x.astype(np.float16).reshape(N, 2, 128, H, TJ, 2).transpose(
            0, 1, 2, 5, 3, 4))

    G = np.array([[1, 0, 0], [0.5, 0.5, 0.5], [0.5, -0.5, 0.5], [0, 0, 1]],
                 np.float64)

    def fold(w, g, bb, m, v):
        inv = np.asarray(g, np.float64) / np.sqrt(np.asarray(v, np.float64) + EPS)
        wp = np.asarray(w, np.float64) * inv[:, None, None, None]
        bp = np.asarray(bb, np.float64) - np.asarray(m, np.float64) * inv
        # wt[pp, kh, plane, ci, o] = sum_kw G[plane, kw] * wp[o, ci*128+pp, kh, kw]
        wt = np.einsum("pw,oihw->hpio", G, wp)          # [3, 4, 256i, 256o]
        wt = wt.reshape(3, 4, 2, 128, 256).transpose(3, 0, 1, 2, 4)
        bt = np.ascontiguousarray(bp.reshape(2, 128).T)
        return np.ascontiguousarray(wt).astype(np.float16), bt.astype(np.float32)

    w1t, b1t = fold(w1, g1, b1, m1, v1)
    w2t, b2t = fold(w2, g2, b2, m2, v2)

    zeros = np.zeros((128, CH * PW), dtype=np.float16)
    idpm = np.concatenate(
        [np.eye(128, dtype=np.float16), -np.eye(128, dtype=np.float16)], axis=1)

    nc = _get_nc()
    in_maps = []
    for c in range(N_CORES):
        in_maps.append({
            "x": np.ascontiguousarray(xp[c * S:(c + 1) * S]),
            "xres": np.ascontiguousarray(xres[c * S:(c + 1) * S]),
            "w1t": w1t, "w2t": w2t, "b1t": b1t, "b2t": b2t,
            "zeros": zeros, "idpm": idpm,
        })

    trace = bool(int(os.environ.get("BASS_KERNEL_TRACE", "0")))
    res = bass_utils.run_bass_kernel_spmd(
        nc, in_maps, core_ids=list(range(N_CORES)), trace=trace)
    LAST_RESULT = res
    out = np.concatenate([r["y"] for r in res.results], axis=0)
    return out


# revision 61
# speedup vs baseline: 1.0883x; 1.0030x over previous
"""ResNet BasicBlock (conv3x3-BN-ReLU-conv3x3-BN-add-ReLU) on 8 Trainium2 cores.

Data-parallel over batch: 32 samples -> 4 per core. Per core, each 3x3 conv
uses 1D Winograd F(2,3) along W: the input is transformed into 4 planes
(t0=d0-d2, t1=d1+d2, t2=d2-d1, t3=d1-d3 over even/odd column pairs), so
each conv is 4 planes x 3 kernel rows x 2 input-channel chunks = 24 f16
matmuls of 392 columns per (output-half, 14-row chunk), one PSUM bank per
plane -- 1.5x fewer tensor cycles than the direct 18-matmul form. The
output transform (y_even=m0+m1+m2, y_odd=m1-m2-m3) runs on the Vector
engine (plane m1 staged to SBUF by a Scalar-engine copy since DVE reads
at most one PSUM operand), fused with the residual add; BN scale is
folded into the conv weights on host, BN bias + ReLU applied by the
Scalar engine. Engine balance: conv1's output strips are re-transformed
for conv2 on GpSimd; input transforms for sample s+2 run on DVE behind
the evacuation ops; conv1 of sample s+1 is emitted before conv2 of
sample s so the tensor queue never waits on the inter-conv transforms.
Measured ~295us vs the 412us direct-conv f16 baseline (tensor engine 96%
occupied within its span; DVE ~252us, GpSimd ~145us, Scalar ~98us).
"""
import os
import sys

for _p in ("/opt/trn_rl_repo", "/root/.axon_site/_ro/trn_rl_repo"):
    if os.path.isdir(_p) and _p not in sys.path:
        sys.path.append(_p)

import numpy as np

EPS = 1e-5

S = 4            # samples per core
C = 256
H = W = 56
PH = 58          # padded rows: image rows 1..56
PW = 60          # padded cols: image cols 2..57, ring zeros at 1,58
FLAT = PH * PW   # 3480
TJ = 28          # column pair tiles
PLANE = PH * TJ  # 1624 elements per transform plane
CH = 14          # output rows per chunk
NCH = 4          # chunks per sample
NCOL = CH * TJ   # 392 moving columns per matmul
N_CORES = 8

_CACHE = {}
LAST_RESULT = None

# residual add: "idmm" = identity matmuls into PSUM, "dve" = vector adds
RESIDUAL = os.environ.get("BASS_KERNEL_RESIDUAL", "dve")


def _build():
    from concourse import bacc
    import concourse.mybir as mybir
    import concourse.tile as tile

    F32 = mybir.dt.float32
    F16 = mybir.dt.float16
    Relu = mybir.ActivationFunctionType.Relu

    nc = bacc.Bacc(None, target_bir_lowering=False)

    x_d = nc.dram_tensor("x", [S, C, PH, PW], F16, kind="ExternalInput")
    # residual, parity-major: [s, ci, p, parity, row, colpair]
    xr_d = nc.dram_tensor("xres", [S, 2, 128, 2, H, TJ], F16, kind="ExternalInput")
    w1_d = nc.dram_tensor("w1t", [128, 3, 4, 2, 256], F16, kind="ExternalInput")
    w2_d = nc.dram_tensor("w2t", [128, 3, 4, 2, 256], F16, kind="ExternalInput")
    b1_d = nc.dram_tensor("b1t", [128, 2], F32, kind="ExternalInput")
    b2_d = nc.dram_tensor("b2t", [128, 2], F32, kind="ExternalInput")
    id_d = nc.dram_tensor("idpm", [128, 256], F16, kind="ExternalInput")
    z_d = nc.dram_tensor("zeros", [128, CH * PW], F16, kind="ExternalInput")
    y_d = nc.dram_tensor("y", [S, C, H, W], F32, kind="ExternalOutput")

    with tile.TileContext(nc) as tc:
        with (
            tc.tile_pool(name="wpool", bufs=1) as wpool,
            tc.tile_pool(name="img", bufs=1) as img,
            tc.tile_pool(name="tmpp", bufs=2) as tmpp,
            tc.tile_pool(name="prep", bufs=2) as prep,
            tc.tile_pool(name="outp", bufs=2) as outp,
            tc.tile_pool(name="ps", bufs=8, space="PSUM") as ps,
        ):
            w_sb = {
                1: wpool.tile([128, 3 * 4 * 2 * 256], F16, name="w1"),
                2: wpool.tile([128, 3 * 4 * 2 * 256], F16, name="w2"),
            }
            b_sb = {
                1: wpool.tile([128, 2], F32, name="b1"),
                2: wpool.tile([128, 2], F32, name="b2"),
            }
            id_sb = wpool.tile([128, 256], F16, name="idpm")
            warm = wpool.tile([128, 4], F32, name="warm")
            xpad = {(b, ci): img.tile([128, FLAT], F16, name=f"xpad{b}_{ci}")
                    for b in range(2) for ci in range(2)}
            xres = {(b, ci): img.tile([128, H * W], F16, name=f"xres{b}_{ci}")
                    for b in range(2) for ci in range(2)}
            # 14-row padded strips for conv1 output (rotating, rings stay 0)
            strip = [img.tile([128, CH * PW], F16, name=f"strip{u}")
                     for u in range(4)]
            tx = {(b, ci): img.tile([128, 4 * PLANE], F16, name=f"tx{b}_{ci}")
                  for b in range(2) for ci in range(2)}
            to = {(b, ci): img.tile([128, 4 * PLANE], F16, name=f"to{b}_{ci}")
                  for b in range(2) for ci in range(2)}

            def pairs(t):
                # [p, row, colpair j, k] with col = 2j + k
                return t.rearrange("p (r j k) -> p r j k", r=PH, k=2)

            def planes(t):
                # [p, plane, row, j]
                return t.rearrange("p (q r j) -> p q r j", q=4, r=PH)

            def tx_plane(src, dst, pl, rows, e):
                a, z = rows
                v = pairs(src)[:, a:z]
                q = planes(dst)[:, pl, a:z, :]
                if pl == 0:
                    e.tensor_sub(q, v[:, :, 0:28, 1], v[:, :, 1:29, 1])
                elif pl == 1:
                    e.tensor_add(q, v[:, :, 1:29, 0], v[:, :, 1:29, 1])
                elif pl == 2:
                    e.tensor_sub(q, v[:, :, 1:29, 1], v[:, :, 1:29, 0])
                else:
                    e.tensor_sub(q, v[:, :, 1:29, 0], v[:, :, 2:30, 0])

            def transform_in(src, dst, eng=None, rows=(0, PH)):
                e = eng or nc.vector
                for pl in range(4):
                    tx_plane(src, dst, pl, rows, e)

            def transform_strip(st, dst, r0, eng):
                # strip rows are padded rows 1+r0 .. 14+r0; steady-state on
                # GpSimd so DVE keeps psum evacuation flowing (DVE for the
                # first, cold-pipeline sample)
                v = st.rearrange("p (r j k) -> p r j k", r=CH, k=2)
                q = planes(dst)[:, :, 1 + r0:1 + r0 + CH, :]
                eng.tensor_sub(q[:, 0], v[:, :, 0:28, 1], v[:, :, 1:29, 1])
                eng.tensor_add(q[:, 1], v[:, :, 1:29, 0], v[:, :, 1:29, 1])
                eng.tensor_sub(q[:, 2], v[:, :, 1:29, 1], v[:, :, 1:29, 0])
                eng.tensor_sub(q[:, 3], v[:, :, 1:29, 0], v[:, :, 2:30, 0])

            def load_x(s, rows=(0, PH)):
                a, z = rows
                for ci in range(2):
                    nc.sync.dma_start(
                        xpad[(s % 2, ci)].rearrange(
                            "p (h w) -> p h w", h=PH)[:, a:z, :],
                        x_d[s, ci * 128:(ci + 1) * 128, a:z, :])

            def load_xres(s):
                b = s % 2
                for ci in range(2):
                    nc.sync.dma_start(
                        xres[(b, ci)].rearrange("p (k r j) -> p k r j",
                                                k=2, r=H),
                        xr_d[s, ci, :, :, :, :])

            # one conv unit: (conv, co, chunk) -> 4 psum planes, 24 matmuls.
            # conv2 also accumulates the residual via identity matmuls:
            # plane0 += I @ x_even, plane3 += (-I) @ x_odd, so the output
            # transform (y_even=m0+m1+m2, y_odd=m1-m2-m3) picks them up free.
            def conv_unit(conv, b, co, c, evac):
                r0 = CH * c
                pts = [ps.tile([128, 512], F32, name="pplane") for _ in range(4)]
                src = tx if conv == 1 else to
                for pl in range(4):
                    for kh in range(3):
                        for ci in range(2):
                            wofs = ((kh * 4 + pl) * 2 + ci) * 256 + co * 128
                            mv = planes(src[(b, ci)])[
                                :, pl, r0 + kh:r0 + kh + CH, :]
                            last = kh == 2 and ci == 1
                            nc.tensor.matmul(
                                pts[pl][:, 0:NCOL],
                                w_sb[conv][:, wofs:wofs + 128],
                                mv,
                                start=(kh == 0 and ci == 0),
                                stop=(last and not
                                      (conv == 2 and RESIDUAL == "idmm"
                                       and pl in (0, 3))),
                            )
                if conv == 2 and RESIDUAL == "idmm":
                    xw = xres[(b, co)].rearrange(
                        "p (k r j) -> p k r j", k=2, r=H)
                    nc.tensor.matmul(
                        pts[0][:, 0:NCOL], id_sb[:, 0:128],
                        xw[:, 0, r0:r0 + CH, :], start=False, stop=True)
                    nc.tensor.matmul(
                        pts[3][:, 0:NCOL], id_sb[:, 128:256],
                        xw[:, 1, r0:r0 + CH, :], start=False, stop=True)
                evac(b, co, c, r0, pts)

            def rj(t):
                return t.rearrange("p (r j) -> p r j", r=CH)

            Copy = mybir.ActivationFunctionType.Copy

            def evac1(b, co, c, r0, pts):
                m = [rj(pts[pl][:, 0:NCOL]) for pl in range(4)]
                pre = prep.tile([128, CH * 56], F16, name="pre")
                pv = pre.rearrange("p (r j k) -> p r j k", r=CH, k=2)
                # DVE reads at most one PSUM operand per op: stage m1 in SBUF
                c1 = rj(tmpp.tile([128, NCOL], F32, name="c1"))
                nc.scalar.activation(c1, m[1], Copy)
                ta = rj(tmpp.tile([128, NCOL], F32, name="ta"))
                nc.vector.tensor_add(ta, c1, m[0])
                nc.vector.tensor_add(pv[:, :, :, 0], ta, m[2])
                tb = rj(tmpp.tile([128, NCOL], F32, name="tb"))
                nc.vector.tensor_sub(tb, c1, m[2])
                nc.vector.tensor_sub(pv[:, :, :, 1], tb, m[3])
                st = strip[(co * NCH + c) % 4]
                sv = st.rearrange("p (r w) -> p r w", r=CH)
                nc.scalar.activation(
                    sv[:, :, 2:58],
                    pre.rearrange("p (r w) -> p r w", r=CH),
                    Relu, bias=b_sb[1][:, co:co + 1])
                # defer the strip->planes transform by one unit so the
                # engine queue never head-of-line blocks on this ACT write
                pending.append((st, to[(b, co)], r0))
                if len(pending) > 1:
                    transform_strip(*pending.pop(0), strip_eng[0])

            pending = []
            strip_eng = [None]

            def flush_strips():
                while pending:
                    transform_strip(*pending.pop(0), strip_eng[0])

            def evac2(s):
                def ev(b, co, c, r0, pts):
                    m = [rj(pts[pl][:, 0:NCOL]) for pl in range(4)]
                    pre = prep.tile([128, CH * 56], F16, name="pre")
                    pv = pre.rearrange("p (r j k) -> p r j k", r=CH, k=2)
                    c1 = rj(tmpp.tile([128, NCOL], F32, name="c1"))
                    nc.scalar.activation(c1, m[1], Copy)
                    ta = rj(tmpp.tile([128, NCOL], F32, name="ta"))
                    nc.vector.tensor_add(ta, c1, m[0])
                    tb = rj(tmpp.tile([128, NCOL], F32, name="tb"))
                    if RESIDUAL == "idmm":
                        nc.vector.tensor_add(pv[:, :, :, 0], ta, m[2])
                        nc.vector.tensor_sub(tb, c1, m[2])
                        nc.vector.tensor_sub(pv[:, :, :, 1], tb, m[3])
                    else:
                        xw = xres[(b, co)].rearrange(
                            "p (k r j) -> p k r j", k=2, r=H)
                        te_ = rj(tmpp.tile([128, NCOL], F16, name="te"))
                        nc.vector.tensor_add(te_, xw[:, 0, r0:r0 + CH, :],
                                             m[2])
                        nc.vector.tensor_add(pv[:, :, :, 0], ta, te_)
                        nc.vector.tensor_sub(tb, c1, m[2])
                        td = rj(tmpp.tile([128, NCOL], F16, name="td"))
                        nc.vector.tensor_sub(td, xw[:, 1, r0:r0 + CH, :],
                                             m[3])
                        nc.vector.tensor_add(pv[:, :, :, 1], tb, td)
                    ot = outp.tile([128, CH * 56], F32, name="ot")
                    nc.scalar.activation(
                        ot.rearrange("p (r w) -> p r w", r=CH),
                        pre.rearrange("p (r w) -> p r w", r=CH),
                        Relu, bias=b_sb[2][:, co:co + 1])
                    nc.sync.dma_start(
                        y_d[s, co * 128:(co + 1) * 128, r0:r0 + CH, :],
                        ot.rearrange("p (r w) -> p r w", r=CH))
                return ev

            def conv_pass(conv, s, evac):
                b = s % 2
                for co in range(2):
                    for c in range(NCH):
                        conv_unit(conv, b, co, c, evac)

            # --- startup ---
            # stage the first chunk's rows so unit 0 starts ~5us in, not 14
            load_x(0, rows=(0, 17))
            nc.sync.dma_start(
                w_sb[1][:, :], w1_d.rearrange("p a b c d -> p (a b c d)"))
            nc.sync.dma_start(b_sb[1][:, :], b1_d[:, :])
            # pull the scalar engine's activation-table load off the critical
            # path (first real ACTIVATE otherwise starts ~12us late)
            nc.vector.memset(warm[:, :], 0.0)
            nc.scalar.activation(warm[:, 0:1], warm[:, 1:2], Relu)
            nc.scalar.activation(warm[:, 2:3], warm[:, 3:4],
                                 mybir.ActivationFunctionType.Copy)
            for pl in range(4):
                for ci in range(2):
                    tx_plane(xpad[(0, ci)], tx[(0, ci)], pl, (0, 17),
                             nc.vector)
            load_x(0, rows=(17, PH))
            load_xres(0)
            nc.sync.dma_start(
                w_sb[2][:, :], w2_d.rearrange("p a b c d -> p (a b c d)"))
            nc.sync.dma_start(b_sb[2][:, :], b2_d[:, :])
            nc.sync.dma_start(id_sb[:, :], id_d[:, :])
            # strips: zero whole tiles once (rings persist; interiors rewritten)
            for st in strip:
                nc.sync.dma_start(st[:, :], z_d[:, :])
            # to planes: rows 0 and 57 (zero-pad rows) are never written later
            for b in range(2):
                for ci in range(2):
                    q = planes(to[(b, ci)])
                    zsrc = z_d[:, 0:4 * TJ].rearrange("p (a c) -> p a c", a=4)
                    nc.sync.dma_start(q[:, :, 0, :], zsrc)
                    nc.sync.dma_start(q[:, :, 57, :], zsrc)
            for pl in range(4):
                for ci in range(2):
                    tx_plane(xpad[(0, ci)], tx[(0, ci)], pl, (17, PH),
                             nc.vector)

            # --- main pipeline: conv1(s+1) emitted before conv2(s) ---
            strip_eng[0] = nc.gpsimd
            conv_pass(1, 0, evac1)
            flush_strips()
            load_x(1)
            load_xres(1)
            for ci in range(2):
                transform_in(xpad[(1, ci)], tx[(1, ci)])
            for s in range(S):
                if s + 1 < S:
                    conv_pass(1, s + 1, evac1)
                    flush_strips()
                    if s + 2 < S:
                        load_x(s + 2)
                        for ci in range(2):
                            transform_in(xpad[((s + 2) % 2, ci)],
                                         tx[((s + 2) % 2, ci)])
                conv_pass(2, s, evac2(s))
                if s + 2 < S:
                    load_xres(s + 2)

    nc.compile()
    return nc


def _get_nc():
    if "nc" not in _CACHE:
        _CACHE["nc"] = _build()
    return _CACHE["nc"]


def kernel(x, w1, g1, b1, m1, v1, w2, g2, b2, m2, v2):
    global LAST_RESULT
    from concourse import bass_utils

    x = np.asarray(x, dtype=np.float32)
    N = x.shape[0]
    xp = np.zeros((N, C, PH, PW), dtype=np.float16)
    xp[:, :, 1:57, 2:58] = x
    # parity-major residual: [n, ci, p, parity, row, colpair]
    xres = np.ascontiguousarray(
        x.astype(np.float16).reshape(N, 2, 128, H, TJ, 2).transpose(
            0, 1, 2, 5, 3, 4))

    G = np.array([[1, 0, 0], [0.5, 0.5, 0.5], [0.5, -0.5, 0.5], [0, 0, 1]],
                 np.float64)

    def fold(w, g, bb, m, v):
        inv = np.asarray(g, np.float64) / np.sqrt(np.asarray(v, np.float64) + EPS)
        wp = np.asarray(w, np.float64) * inv[:, None, None, None]
        bp = np.asarray(bb, np.float64) - np.asarray(m, np.float64) * inv
        # wt[pp, kh, plane, ci, o] = sum_kw G[plane, kw] * wp[o, ci*128+pp, kh, kw]
        wt = np.einsum("pw,oihw->hpio", G, wp)          # [3, 4, 256i, 256o]
        wt = wt.reshape(3, 4, 2, 128, 256).transpose(3, 0, 1, 2, 4)
        bt = np.ascontiguousarray(bp.reshape(2, 128).T)
        return np.ascontiguousarray(wt).astype(np.float16), bt.astype(np.float32)

    w1t, b1t = fold(w1, g1, b1, m1, v1)
    w2t, b2t = fold(w2, g2, b2, m2, v2)

    zeros = np.zeros((128, CH * PW), dtype=np.float16)
    idpm = np.concatenate(
        [np.eye(128, dtype=np.float16), -np.eye(128, dtype=np.float16)], axis=1)

    nc = _get_nc()
    in_maps = []
    for c in range(N_CORES):
        in_maps.append({
            "x": np.ascontiguousarray(xp[c * S:(c + 1) * S]),
            "xres": np.ascontiguousarray(xres[c * S:(c + 1) * S]),
            "w1t": w1t, "w2t": w2t, "b1t": b1t, "b2t": b2t,
            "zeros": zeros, "idpm": idpm,
        })

    trace = bool(int(os.environ.get("BASS_KERNEL_TRACE", "0")))
    res = bass_utils.run_bass_kernel_spmd(
        nc, in_maps, core_ids=list(range(N_CORES)), trace=trace)
    LAST_RESULT = res
    out = np.concatenate([r["y"] for r in res.results], axis=0)
    return out
